# revision 5
# baseline (speedup 1.0000x reference)
"""CRF Viterbi decode (B=1024, T=1024, N=32) on 8 TRN2 NeuronCores — v3.

Meet-in-the-middle: a forward Viterbi recursion over t=[0, M] and a
backward (future-score) relay over t=[1023, M] run as two interleaved
DVE chains, hiding each other's dependency bubbles; the per-step logit
adds run on the Pool engine.  At t=M the two sides meet:
argmax(fwd + bwd) anchors the path, after which two independent
backtrace walks (down from M over fwd states, up from M over stored
backward u-vectors) interleave to hide the DVE->PE->DVE latency.

The backward relay's logits are biased by -BIAS (expected per-step max
gain) so backward scores stay near zero magnitude; the uniform shift
never changes an argmax but keeps f32 rounding noise ~1e-4, far below
typical decision gaps.  The forward side is bit-exact to the reference.
"""
import sys
sys.path.insert(0, "/opt/trn_rl_repo")

import numpy as np

import concourse.bass as bass
import concourse.bacc as bacc
import concourse.mybir as mybir
import concourse.tile as tile
from concourse.bass_utils import run_bass_kernel_spmd

F32 = mybir.dt.float32
I32 = mybir.dt.int32
I8 = mybir.dt.int8
AX = mybir.AxisListType
OP = mybir.AluOpType

B, T, N = 1024, 1024, 32
PB = 128
NCORES = 8
BIG = 1.0e6
M = 512           # meet point
BIAS = 2.1        # backward-relay logit bias
CH = 16           # DMA chunk (time steps)

_ops_cache = {}
_nc_cache = {}
_last_exec_ns = [None]


def register_custom_ops():
    if _ops_cache:
        return _ops_cache["BT32"]
    from concourse.dve_spec import (
        Spec, Src0, Src1, AluOp, lower, Idx, scan, Scan, MaxNeg,
    )
    from concourse.dve_ops import DveOp, OPS, has_src1
    from concourse.dve_uop import DveOpSpec, AluInp
    import concourse.dve_ops as dom

    def make(name, spec, subdim, patch=None, opcfg=None):
        for o in OPS:
            if o.name == name:
                return o
        OPS_len = len(OPS)
        dom._SUB_OPCODE_FOR_NAME[name] = dom._CUSTOM_DVE_ROW_BASE + OPS_len
        assert dom._SUB_OPCODE_FOR_NAME[name] < 0x20
        shas = {}
        for ver in ("v3", "v4"):
            uops = lower(spec, ver=ver)
            if patch is not None:
                patch(uops)
            kw = {} if opcfg is None else {"op": opcfg}
            s = DveOpSpec(name=name, opcode=dom.get_dve_sub_opcode(name),
                          uops=uops, rd1_en=has_src1(spec), **kw)
            shas[ver] = s.sha(ver)
            dom._COMPILE_CACHE[(name, ver)] = s
        op = DveOp(name, spec, subdim=subdim, uops_sha=shas)
        OPS.append(op)
        dom.CUSTOM_DVE_SPECS[name] = spec
        return op

    def make_segmax():
        """Segmented running-max of (Src0 + Src1), reset at each subdim
        (row) boundary."""
        def ref(in0, in1, c0, c1, c2):
            x = (in0 + in1).astype(np.float32)
            r = np.maximum.accumulate(x, axis=-1)
            return r, None

        sc = Scan(AluOp.MAX, Src0 + Src1, _subdim_step=MaxNeg)
        spec = Spec(body=sc, reference=ref)

        def patch(uops):
            assert len(uops) == 3, f"expected [seed, steady, step], got {len(uops)}"
            steady, step = uops[1], uops[2]
            dp = steady.datapath_config[1]
            dp.op = AluOp.MAX
            dp.alu_src0 = AluInp.CURR_ALU_OUT
            dp.alu_src1 = AluInp.PREV_ALU_OUT
            dp = step.datapath_config[1]
            dp.op = AluOp.BYPASS
            dp.alu_src0 = AluInp.PREV_ALU_OUT
            dp.alu_src1 = AluInp.PREV_ALU_OUT

        return make("CRF_SEGMAX", spec, subdim=True, patch=patch)

    FMAX = np.float32(3.4028235e38)

    def ref3(in0, in1, c0, c1, c2):
        P, K = in0.shape
        x = (in0 + in1).astype(np.float32)
        r = np.maximum.accumulate(x, axis=1)
        m = ((x == r).astype(np.float32) * np.arange(K, dtype=np.float32)[None, :])
        return m, m.max(axis=1, initial=-FMAX).reshape(P, 1)

    from concourse.dve_spec import eq
    _x3 = Src0 + Src1
    spec3 = Spec(body=eq(_x3, scan(AluOp.MAX, _x3)) * Idx, accum=AluOp.MAX,
                 reference=ref3)

    op3 = make("CRF_BT32", spec3, subdim=False)
    opF = make_segmax()

    # Fused onehot+transpose for the walk: transpose_mode applies a 32x32
    # block transpose to SRC0 only; src1 streams in the output domain
    # (verified on device).  out[32b+c, j] = (in0[32b+j, *] == in1[32b+c, j]).
    from concourse.dve_uop import OpConfig, TransposeMode

    def _blockT(x):
        o = np.empty_like(x)
        for b in range(0, x.shape[0], 32):
            o[b:b + 32, :] = x[b:b + 32, :].T
        return o

    def refT(in0, in1, c0, c1, c2):
        return (_blockT(in0) == in1).astype(np.float32), None

    specT = Spec(body=eq(Src0, Src1), reference=refT)
    opT = make("CRF_OHT2", specT, subdim=False,
               opcfg=OpConfig(transpose_mode=TransposeMode.TRANSPOSE))

    _ops_cache["BT32"] = op3
    _ops_cache["SEGMAX"] = opF
    _ops_cache["OHT2"] = opT
    return op3


def build_nc(Tn):
    assert Tn == T
    register_custom_ops()
    op3 = _ops_cache["BT32"]
    opF = _ops_cache["SEGMAX"]
    opT = _ops_cache["OHT2"]

    NFWD = M          # fwd steps t=1..M
    NBWD = T - 1 - M  # bwd segmax steps t=T-2..M (count 511)
    NU = T - 1 - M    # u slots for t in [M+1, T-1]

    nc = bacc.Bacc("TRN2", target_bir_lowering=False, debug=False,
                   num_devices=NCORES)

    logits = nc.dram_tensor("logits", [PB, T, N], F32, kind="ExternalInput")
    logitsB_d = nc.dram_tensor("logitsB", [PB, T - M, N], F32,
                               kind="ExternalInput")  # biased, slot(t)=t-M
    trepF_d = nc.dram_tensor("trepF", [PB, N * N], F32, kind="ExternalInput")
    trepB_d = nc.dram_tensor("trepB", [PB, N * N], F32, kind="ExternalInput")
    meq_d = nc.dram_tensor("meq", [PB, M + 1], F32, kind="ExternalInput")
    keep_d = nc.dram_tensor("keep", [PB, T], F32, kind="ExternalInput")
    mlt_d = nc.dram_tensor("mlt", [PB, T], F32, kind="ExternalInput")
    mrw_d = nc.dram_tensor("mrw", [PB, M + 1], I8, kind="ExternalInput")
    mrwf_d = nc.dram_tensor("mrwf", [PB, M + 1], F32, kind="ExternalInput")
    mrwc_d = nc.dram_tensor("mrwc", [PB, M + 1], F32, kind="ExternalInput")
    irev_d = nc.dram_tensor("irev", [PB, N], F32, kind="ExternalInput")
    irevp_d = nc.dram_tensor("irevP", [PB, N], F32, kind="ExternalInput")
    trevwF_d = nc.dram_tensor("trevwF", [PB, N], F32, kind="ExternalInput")
    trevwB_d = nc.dram_tensor("trevwB", [PB, N], F32, kind="ExternalInput")
    out_d = nc.dram_tensor("out", [PB, T], I32, kind="ExternalOutput")

    nfch = (M + CH) // CH          # fwd chunks cover t=0..M (33, last partial)
    nbch = (T - M) // CH           # bwd chunks cover t=M..T-1 biased (32)

    with tile.TileContext(nc) as tc:
        with (
            tc.tile_pool(name="consts", bufs=1) as cpool,
            tc.tile_pool(name="states", bufs=1) as spool,
            tc.tile_pool(name="big", bufs=1) as bpool,
            tc.tile_pool(name="lchF", bufs=2) as lpoolF,
            tc.tile_pool(name="lchB", bufs=2) as lpoolB,
            tc.tile_pool(name="small", bufs=1) as mpool,
            tc.tile_pool(name="psum", bufs=1, space="PSUM") as ppool,
        ):
            trepF = cpool.tile([PB, N * N], F32, tag="trepF")
            trepB = cpool.tile([PB, N * N], F32, tag="trepB")
            meq = cpool.tile([PB, M + 1], F32, tag="meq")
            keep = cpool.tile([PB, T], F32, tag="keep")
            mlt = cpool.tile([PB, T], F32, tag="mlt")
            mrw = cpool.tile([PB, M + 1], I8, tag="mrw")
            mrwf = cpool.tile([PB, M + 1], F32, tag="mrwf")
            mrwc = cpool.tile([PB, M + 1], F32, tag="mrwc")
            irev = cpool.tile([PB, N], F32, tag="irev")
            irevP = cpool.tile([PB, N], F32, tag="irevP")
            trevwF = cpool.tile([PB, N], F32, tag="trevwF")
            trevwB = cpool.tile([PB, N], F32, tag="trevwB")

            statesF = spool.tile([PB, (M + 1) * N], F32, tag="statesF")
            uB = spool.tile([PB, NU * N], F32, tag="uB")

            scoresF = bpool.tile([PB, N * N], F32, tag="scoresF")
            scoresB = bpool.tile([PB, N * N], F32, tag="scoresB")
            tags = bpool.tile([PB, T], F32, tag="tags")
            outi = bpool.tile([PB, T], I32, tag="outi")
            accS = bpool.tile([PB, CH * N], F32, tag="accS")
            rsc = bpool.tile([PB, CH * N], F32, tag="rsc")
            rrw = bpool.tile([PB, 64 * N], F32, tag="rrw")

            beta0 = mpool.tile([PB, N], F32, tag="beta0")
            snap = mpool.tile([PB, N], F32, tag="snap")
            lastt = mpool.tile([PB, 1], F32, tag="lastt")
            eqs = mpool.tile([PB, N], F32, tag="eqs")
            red = mpool.tile([PB, N], F32, tag="red")
            m1 = mpool.tile([PB, 1], F32, tag="m1")
            onehot = mpool.tile([PB, N], F32, tag="onehot")
            bigoh = mpool.tile([PB, N], F32, tag="bigoh")
            mbeta = mpool.tile([PB, N], F32, tag="mbeta")
            msc = mpool.tile([PB, N], F32, tag="msc")
            onehotF2 = [mpool.tile([PB, N], F32, name=f"onehotF{x}")
                        for x in range(2)]
            onehotTF2 = [mpool.tile([PB, N], F32, name=f"onehotTF{x}")
                         for x in range(2)]
            redF = mpool.tile([PB, N], F32, tag="redF")
            onehotB2 = [mpool.tile([PB, N], F32, name=f"onehotB{x}")
                        for x in range(2)]
            onehotTB2 = [mpool.tile([PB, N], F32, name=f"onehotTB{x}")
                         for x in range(2)]
            redB = mpool.tile([PB, N], F32, tag="redB")
            tselF2 = [ppool.tile([PB, N], F32, name=f"tselF{x}")
                      for x in range(2)]
            tselB2 = [ppool.tile([PB, N], F32, name=f"tselB{x}")
                      for x in range(2)]

            # ---- first chunks gate the recursion start: issue them first ----
            ltF0 = lpoolF.tile([PB, CH * N], F32, tag="lchF")
            nc.sync.dma_start(
                out=ltF0[:].rearrange("p (t v) -> p t v", v=N),
                in_=logits.ap()[:, 0:CH, :])
            ltB0 = lpoolB.tile([PB, CH * N], F32, tag="lchB")
            nc.sync.dma_start(
                out=ltB0[:].rearrange("p (t v) -> p t v", v=N),
                in_=logitsB_d.ap()[:, T - M - CH:T - M, :])  # t in [1008,1023]
            nc.sync.dma_start(out=trepF[:], in_=trepF_d.ap())
            nc.sync.dma_start(out=trepB[:], in_=trepB_d.ap())
            nc.sync.dma_start(out=keep[:], in_=keep_d.ap())
            nc.sync.dma_start(out=meq[:], in_=meq_d.ap())
            nc.sync.dma_start(out=mlt[:], in_=mlt_d.ap())
            nc.sync.dma_start(out=mrw[:], in_=mrw_d.ap())
            nc.sync.dma_start(out=mrwf[:], in_=mrwf_d.ap())
            nc.sync.dma_start(out=mrwc[:], in_=mrwc_d.ap())
            nc.sync.dma_start(out=irev[:], in_=irev_d.ap())
            nc.sync.dma_start(out=irevP[:], in_=irevp_d.ap())
            nc.sync.dma_start(out=trevwF[:], in_=trevwF_d.ap())
            nc.sync.dma_start(out=trevwB[:], in_=trevwB_d.ap())

            trepF3 = trepF[:].rearrange("p (c v) -> p c v", v=N)
            trepB3 = trepB[:].rearrange("p (c v) -> p c v", v=N)
            scF3 = scoresF[:].rearrange("p (c v) -> p c v", v=N)
            scB3 = scoresB[:].rearrange("p (c v) -> p c v", v=N)
            statesF3 = statesF[:].rearrange("p (t v) -> p t v", v=N)
            uB3 = uB[:].rearrange("p (t v) -> p t v", v=N)
            accS3 = accS[:].rearrange("p (t v) -> p t v", v=N)
            rsc3 = rsc[:].rearrange("p (t v) -> p t v", v=N)

            nc.vector.memset(beta0[:], 0.0)
            nc.gpsimd.memset(accS[:], 0.0)

            # statesF[0] = logits[0]
            nc.vector.tensor_copy(out=statesF[:, 0:N], in_=ltF0[:, 0:N])
            # u[T-1] = beta0*keep[T-1] + logitsB[T-1]  (Pool: 2x tensor_tensor;
            # TensorScalarPtr is rejected on Pool by codegen)
            keepb_last = keep[:, T - 1:T].rearrange("p (a b) -> p a b", a=1) \
                .to_broadcast((PB, N, 1))
            u_last = uB[:, (NU - 1) * N:NU * N].rearrange("p (c o) -> p c o", o=1)
            nc.gpsimd.tensor_tensor(
                out=u_last, in0=beta0[:].rearrange("p (c o) -> p c o", o=1),
                in1=keepb_last, op=OP.mult)
            nc.gpsimd.tensor_tensor(
                out=u_last, in0=u_last,
                in1=ltB0[:, (CH - 1) * N:CH * N].rearrange("p (c o) -> p c o", o=1),
                op=OP.add)

            # ---------------- phase R: interleaved recursions ----------------
            ltF, ltB = ltF0, ltB0
            ltB_next = None
            # u slot for t: t - (M+1); logitsB slot for t: t - M
            for i in range(NFWD):          # i = 0..511
                tf = i + 1                 # fwd computes statesF[tf]
                tb = T - 2 - i             # bwd computes beta_tb (i<NBWD)
                # chunk management
                if tf % CH == 0 and tf < M:
                    ltF = lpoolF.tile([PB, CH * N], F32, tag="lchF")
                    nc.sync.dma_start(
                        out=ltF[:].rearrange("p (t v) -> p t v", v=N),
                        in_=logits.ap()[:, tf:tf + CH, :])
                elif tf == M:
                    ltF = lpoolF.tile([PB, CH * N], F32, tag="lchF")
                    nc.sync.dma_start(
                        out=ltF[:, 0:N].rearrange("p (t v) -> p t v", v=N),
                        in_=logits.ap()[:, M:M + 1, :])
                # bwd chunk k = i//CH + 1 prefetched a full chunk early
                # (issued at i ≡ 0 mod CH, first consumed at i ≡ CH-1)
                if i % CH == 0 and i // CH + 1 < nbch:
                    k = i // CH + 1
                    ltB_next = lpoolB.tile([PB, CH * N], F32, tag="lchB")
                    nc.sync.dma_start(
                        out=ltB_next[:].rearrange("p (t v) -> p t v", v=N),
                        in_=logitsB_d.ap()[:, T - M - CH * (k + 1):
                                           T - M - CH * k, :])
                if (i + 1) % CH == 0:
                    # tb at this i (= T-CH*k-1) already reads the new chunk
                    ltB = ltB_next

                # ---- fwd step ----
                sprev_b = statesF[:, (tf - 1) * N:tf * N] \
                    .rearrange("p (o v) -> p o v", o=1).to_broadcast((PB, N, N))
                nc.vector._custom_dve(opF, out=scF3, in0=sprev_b, in1=trepF3)
                nc.gpsimd.tensor_tensor(
                    out=statesF[:, tf * N:(tf + 1) * N]
                        .rearrange("p (c o) -> p c o", o=1),
                    in0=scF3[:, :, N - 1:N],
                    in1=ltF[:, (tf % CH) * N:(tf % CH + 1) * N]
                        .rearrange("p (c o) -> p c o", o=1),
                    op=OP.add)

                # ---- bwd step ----
                if i < NBWD:
                    usl1 = tb + 1 - (M + 1)
                    u_next_b = uB[:, usl1 * N:(usl1 + 1) * N] \
                        .rearrange("p (o v) -> p o v", o=1) \
                        .to_broadcast((PB, N, N))
                    nc.vector._custom_dve(opF, out=scB3, in0=u_next_b,
                                          in1=trepB3)
                    if tb >= M + 1:
                        usl0 = tb - (M + 1)
                        u_out = uB[:, usl0 * N:(usl0 + 1) * N] \
                            .rearrange("p (c o) -> p c o", o=1)
                        keepb = keep[:, tb:tb + 1] \
                            .rearrange("p (a b) -> p a b", a=1) \
                            .to_broadcast((PB, N, 1))
                        nc.gpsimd.tensor_tensor(
                            out=u_out, in0=scB3[:, :, N - 1:N], in1=keepb,
                            op=OP.mult)
                        nc.gpsimd.tensor_tensor(
                            out=u_out, in0=u_out,
                            in1=ltB[:, (tb % CH) * N:(tb % CH + 1) * N]
                                .rearrange("p (c o) -> p c o", o=1),
                            op=OP.add)

                # ---- snapshot accumulation (Pool), spread in 4-row pieces
                # one chunk behind the recursion to avoid queue bursts ----
                if tf > CH and (tf - 1) % 4 == 0:
                    # piece r of chunk ending at the last CH boundary
                    bnd = ((tf - 1) // CH) * CH     # last boundary < tf
                    r = ((tf - 1) % CH) // 4
                    t0 = bnd - CH + r * 4
                    if t0 + 4 <= bnd:
                        meqb = meq[:, t0:t0 + 4] \
                            .rearrange("p (t o) -> p t o", o=1) \
                            .to_broadcast((PB, 4, N))
                        nc.gpsimd.tensor_tensor(
                            out=rsc3[:, r * 4:r * 4 + 4, :],
                            in0=statesF3[:, t0:t0 + 4, :], in1=meqb,
                            op=OP.mult)
                        nc.gpsimd.tensor_tensor(
                            out=accS3[:, r * 4:r * 4 + 4, :],
                            in0=accS3[:, r * 4:r * 4 + 4, :],
                            in1=rsc3[:, r * 4:r * 4 + 4, :], op=OP.add)

            # remaining snapshot rows: the in-loop pieces covered t in
            # [0, M-CH); add [M-CH, M] (CH+1 rows) here.
            meqb_m = meq[:, M - CH:M + 1].rearrange("p (t o) -> p t o", o=1) \
                .to_broadcast((PB, CH + 1, N))
            rscw = rsc[:].rearrange("p (t v) -> p t v", v=N)
            accw = accS[:].rearrange("p (t v) -> p t v", v=N)
            nc.gpsimd.tensor_tensor(
                out=rscw[:, 0:CH, :], in0=statesF3[:, M - CH:M, :],
                in1=meqb_m[:, 0:CH, :], op=OP.mult)
            nc.gpsimd.tensor_tensor(
                out=accw[:, 0:CH, :], in0=accw[:, 0:CH, :],
                in1=rscw[:, 0:CH, :], op=OP.add)
            nc.gpsimd.tensor_tensor(
                out=rscw[:, 0:1, :], in0=statesF3[:, M:M + 1, :],
                in1=meqb_m[:, CH:CH + 1, :], op=OP.mult)
            nc.gpsimd.tensor_tensor(
                out=accw[:, 0:1, :], in0=accw[:, 0:1, :],
                in1=rscw[:, 0:1, :], op=OP.add)

            # ---------------- phase M: snap -> last_tag -> rewrite -> meet ----
            acc_vt = accS[:].rearrange("p (t v) -> p v t", v=N)
            nc.vector.tensor_reduce(out=snap[:], in_=acc_vt, axis=AX.X,
                                    op=OP.add)
            nc.vector.tensor_reduce(out=m1[:], in_=snap[:], axis=AX.X, op=OP.max)
            nc.vector.tensor_scalar(
                out=eqs[:], in0=snap[:], scalar1=m1[:], scalar2=None,
                op0=OP.is_equal)
            nc.vector.tensor_tensor(out=red[:], in0=eqs[:], in1=irev[:],
                                    op=OP.mult)
            nc.vector.tensor_reduce(out=lastt[:], in_=red[:], axis=AX.X,
                                    op=OP.max)

            # BIG-rewrite of statesF rows t in [len-1, M]; top rows sync via
            # copy_predicated, lower rows async on Pool during the walk.
            RS = M - 64
            RCH = 64
            nc.vector.tensor_scalar(
                out=onehot[:], in0=irev[:], scalar1=lastt[:], scalar2=None,
                op0=OP.is_equal)
            nc.vector.tensor_scalar(
                out=bigoh[:], in0=onehot[:], scalar1=BIG, scalar2=None,
                op0=OP.mult)
            mrwb = mrw[:, RS:M + 1].rearrange("p (t o) -> p t o", o=1) \
                .to_broadcast((PB, M + 1 - RS, N))
            bigohb = bigoh[:].rearrange("p (o v) -> p o v", o=1) \
                .to_broadcast((PB, M + 1 - RS, N))
            nc.vector.copy_predicated(out=statesF3[:, RS:M + 1, :], mask=mrwb,
                                      data=bigohb)
            rrw3 = rrw[:].rearrange("p (t v) -> p t v", v=N)
            bigohc = bigoh[:].rearrange("p (o v) -> p o v", o=1) \
                .to_broadcast((PB, RCH, N))
            for t0 in range(RS - RCH, -1, -RCH):
                stc = statesF3[:, t0:t0 + RCH, :]
                mc = mrwc[:, t0:t0 + RCH] \
                    .rearrange("p (t o) -> p t o", o=1).to_broadcast((PB, RCH, N))
                mf = mrwf[:, t0:t0 + RCH] \
                    .rearrange("p (t o) -> p t o", o=1).to_broadcast((PB, RCH, N))
                nc.gpsimd.tensor_tensor(out=stc, in0=stc, in1=mc, op=OP.mult)
                nc.gpsimd.tensor_tensor(out=rrw3, in0=bigohc, in1=mf,
                                        op=OP.mult)
                nc.gpsimd.tensor_tensor(out=stc, in0=stc, in1=rrw3, op=OP.add)

            # meet: tags[M] = argmax-enc(statesF[M] + beta_M * keep[M])
            nc.vector.tensor_scalar(
                out=mbeta[:].rearrange("p (c o) -> p c o", o=1),
                in0=scB3[:, :, N - 1:N],
                scalar1=keep[:, M:M + 1], scalar2=None, op0=OP.mult)
            nc.vector.tensor_tensor(out=msc[:], in0=statesF[:, M * N:(M + 1) * N],
                                    in1=mbeta[:], op=OP.add)
            nc.vector.tensor_reduce(out=m1[:], in_=msc[:], axis=AX.X, op=OP.max)
            nc.vector.tensor_scalar(
                out=eqs[:], in0=msc[:], scalar1=m1[:], scalar2=None,
                op0=OP.is_equal)
            nc.vector.tensor_tensor(out=red[:], in0=eqs[:], in1=irev[:],
                                    op=OP.mult)
            nc.vector.tensor_reduce(out=tags[:, M:M + 1], in_=red[:],
                                    axis=AX.X, op=OP.max)

            # ---------------- phase W: two interleaved walks ----------------
            for j in range(M):
                tfw = M - 1 - j            # fwd walk target position
                tbw = M + 1 + j            # bwd walk target position
                par = j % 2
                onehotF, onehotTF, tselF = (onehotF2[par], onehotTF2[par],
                                            tselF2[par])
                onehotB, onehotTB, tselB = (onehotB2[par], onehotTB2[par],
                                            tselB2[par])
                # fwd walk step: fused compare + src0 block-transpose
                nc.vector._custom_dve(
                    opT, out=onehotTF[:],
                    in0=tags[:, tfw + 1:tfw + 2].to_broadcast((PB, N)),
                    in1=irevP[:])
                for blk in range(4):
                    nc.tensor.matmul(
                        out=tselF[blk * N:(blk + 1) * N, :],
                        lhsT=onehotTF[blk * N:(blk + 1) * N, :],
                        rhs=trevwF[blk * N:(blk + 1) * N, :],
                        start=True, stop=True,
                        tile_position=(blk * N, blk * N))
                nc.vector._custom_dve(
                    op3, out=redF[:], in0=tselF[:],
                    in1=statesF[:, tfw * N:(tfw + 1) * N][:, ::-1],
                    accum_out=tags[:, tfw:tfw + 1])
                # bwd walk step
                if tbw <= T - 1:
                    nc.vector._custom_dve(
                        opT, out=onehotTB[:],
                        in0=tags[:, tbw - 1:tbw].to_broadcast((PB, N)),
                        in1=irevP[:])
                    for blk in range(4):
                        nc.tensor.matmul(
                            out=tselB[blk * N:(blk + 1) * N, :],
                            lhsT=onehotTB[blk * N:(blk + 1) * N, :],
                            rhs=trevwB[blk * N:(blk + 1) * N, :],
                            start=True, stop=True,
                            tile_position=(blk * N, blk * N))
                    usl = tbw - (M + 1)
                    nc.vector._custom_dve(
                        op3, out=redB[:], in0=tselB[:],
                        in1=uB[:, usl * N:(usl + 1) * N][:, ::-1],
                        accum_out=tags[:, tbw:tbw + 1])

            # ---------------- decode + mask + output ----------------
            nc.vector.tensor_scalar(
                out=tags[:], in0=tags[:], scalar1=-1.0, scalar2=31.0,
                op0=OP.mult, op1=OP.add)
            nc.vector.tensor_tensor(out=outi[:], in0=tags[:], in1=mlt[:],
                                    op=OP.mult)
            nc.sync.dma_start(out=out_d.ap(), in_=outi[:])

    nc.compile()
    return nc


def make_inputs_for_core(logits_shard, lens_shard):
    Tmat = _tmat_holder[0]
    lens = lens_shard.astype(np.int64)[:, None]
    tcolM = np.arange(M + 1)[None, :]
    tcolT = np.arange(T)[None, :]
    meq = (lens == (tcolM + 1)).astype(np.float32)
    keep = (lens - 1 != tcolT).astype(np.float32)
    mlt = (tcolT < lens).astype(np.float32)
    mrw = (tcolM >= (lens - 1)).astype(np.int8)
    irev = (31.0 - np.arange(N, dtype=np.float32))[None, :]
    irevP = np.ascontiguousarray(
        np.broadcast_to((31.0 - (np.arange(PB) % N))[:, None], (PB, N)),
        dtype=np.float32)
    rep = lambda a: np.ascontiguousarray(
        np.broadcast_to(a, (PB, a.shape[1])), dtype=np.float32)
    trepF = np.ascontiguousarray(Tmat.T).reshape(1, N * N)
    trepB = np.ascontiguousarray(Tmat).reshape(1, N * N)
    logitsB = np.ascontiguousarray(
        logits_shard[:, M:, :] - np.float32(BIAS), dtype=np.float32)
    return {
        "logits": np.ascontiguousarray(logits_shard, dtype=np.float32),
        "logitsB": logitsB,
        "trepF": rep(trepF),
        "trepB": rep(trepB),
        "meq": np.ascontiguousarray(meq, dtype=np.float32),
        "keep": np.ascontiguousarray(keep, dtype=np.float32),
        "mlt": np.ascontiguousarray(mlt, dtype=np.float32),
        "mrw": np.ascontiguousarray(mrw, dtype=np.int8),
        "mrwf": np.ascontiguousarray(mrw, dtype=np.float32),
        "mrwc": np.ascontiguousarray(1 - mrw, dtype=np.float32),
        "irev": rep(irev),
        "irevP": irevP,
        "trevwF": np.ascontiguousarray(
            np.tile(Tmat[::-1, :].T, (4, 1)), dtype=np.float32),
        "trevwB": np.ascontiguousarray(
            np.tile(Tmat[:, ::-1], (4, 1)), dtype=np.float32),
    }


_tmat_holder = [None]


def last_exec_time_ns():
    return _last_exec_ns[0]


def kernel(logits, transitions, sequence_lengths, _trace=False):
    logits = np.asarray(logits, dtype=np.float32)
    Tmat = np.asarray(transitions, dtype=np.float32)
    lens = np.asarray(sequence_lengths)
    Bn, Tn, Nn = logits.shape
    assert Nn == N and Bn % NCORES == 0
    _tmat_holder[0] = Tmat

    if Tn not in _nc_cache:
        _nc_cache[Tn] = build_nc(Tn)
    nc = _nc_cache[Tn]

    in_maps = []
    for i in range(NCORES):
        sl = slice(i * PB, (i + 1) * PB)
        in_maps.append(make_inputs_for_core(logits[sl], lens[sl]))

    kw = {}
    if _trace:
        kw = dict(trace=True, trace_cores=[0])
    res = run_bass_kernel_spmd(nc, in_maps, core_ids=list(range(NCORES)), **kw)
    _last_exec_ns[0] = getattr(res, "exec_time_ns", None)

    out = np.concatenate([res.results[i]["out"] for i in range(NCORES)], axis=0)
    return out.astype(np.int32)


# revision 6
# speedup vs baseline: 1.0004x; 1.0004x over previous
"""CRF Viterbi decode (B=1024, T=1024, N=32) on 8 TRN2 NeuronCores — v3.

Meet-in-the-middle: a forward Viterbi recursion over t=[0, M] and a
backward (future-score) relay over t=[1023, M] run as two interleaved
DVE chains, hiding each other's dependency bubbles; the per-step logit
adds run on the Pool engine.  At t=M the two sides meet:
argmax(fwd + bwd) anchors the path, after which two independent
backtrace walks (down from M over fwd states, up from M over stored
backward u-vectors) interleave to hide the DVE->PE->DVE latency.

The backward relay's logits are biased by -BIAS (expected per-step max
gain) so backward scores stay near zero magnitude; the uniform shift
never changes an argmax but keeps f32 rounding noise ~1e-4, far below
typical decision gaps.  The forward side is bit-exact to the reference.
"""
import sys
sys.path.insert(0, "/opt/trn_rl_repo")

import numpy as np

import concourse.bass as bass
import concourse.bacc as bacc
import concourse.mybir as mybir
import concourse.tile as tile
from concourse.bass_utils import run_bass_kernel_spmd

F32 = mybir.dt.float32
I32 = mybir.dt.int32
I8 = mybir.dt.int8
AX = mybir.AxisListType
OP = mybir.AluOpType

B, T, N = 1024, 1024, 32
PB = 128
NCORES = 8
BIG = 1.0e6
M = 512           # meet point
BIAS = 2.1        # backward-relay logit bias
CH = 16           # DMA chunk (time steps)

_ops_cache = {}
_nc_cache = {}
_last_exec_ns = [None]


def register_custom_ops():
    if _ops_cache:
        return _ops_cache["BT32"]
    from concourse.dve_spec import (
        Spec, Src0, Src1, AluOp, lower, Idx, scan, Scan, MaxNeg,
    )
    from concourse.dve_ops import DveOp, OPS, has_src1
    from concourse.dve_uop import DveOpSpec, AluInp
    import concourse.dve_ops as dom

    def make(name, spec, subdim, patch=None, opcfg=None):
        for o in OPS:
            if o.name == name:
                return o
        OPS_len = len(OPS)
        dom._SUB_OPCODE_FOR_NAME[name] = dom._CUSTOM_DVE_ROW_BASE + OPS_len
        assert dom._SUB_OPCODE_FOR_NAME[name] < 0x20
        shas = {}
        for ver in ("v3", "v4"):
            uops = lower(spec, ver=ver)
            if patch is not None:
                patch(uops)
            kw = {} if opcfg is None else {"op": opcfg}
            s = DveOpSpec(name=name, opcode=dom.get_dve_sub_opcode(name),
                          uops=uops, rd1_en=has_src1(spec), **kw)
            shas[ver] = s.sha(ver)
            dom._COMPILE_CACHE[(name, ver)] = s
        op = DveOp(name, spec, subdim=subdim, uops_sha=shas)
        OPS.append(op)
        dom.CUSTOM_DVE_SPECS[name] = spec
        return op

    def make_segmax():
        """Segmented running-max of (Src0 + Src1), reset at each subdim
        (row) boundary."""
        def ref(in0, in1, c0, c1, c2):
            x = (in0 + in1).astype(np.float32)
            r = np.maximum.accumulate(x, axis=-1)
            return r, None

        sc = Scan(AluOp.MAX, Src0 + Src1, _subdim_step=MaxNeg)
        spec = Spec(body=sc, reference=ref)

        def patch(uops):
            assert len(uops) == 3, f"expected [seed, steady, step], got {len(uops)}"
            steady, step = uops[1], uops[2]
            dp = steady.datapath_config[1]
            dp.op = AluOp.MAX
            dp.alu_src0 = AluInp.CURR_ALU_OUT
            dp.alu_src1 = AluInp.PREV_ALU_OUT
            dp = step.datapath_config[1]
            dp.op = AluOp.BYPASS
            dp.alu_src0 = AluInp.PREV_ALU_OUT
            dp.alu_src1 = AluInp.PREV_ALU_OUT

        return make("CRF_SEGMAX", spec, subdim=True, patch=patch)

    FMAX = np.float32(3.4028235e38)

    def ref3(in0, in1, c0, c1, c2):
        P, K = in0.shape
        x = (in0 + in1).astype(np.float32)
        r = np.maximum.accumulate(x, axis=1)
        m = ((x == r).astype(np.float32) * np.arange(K, dtype=np.float32)[None, :])
        return m, m.max(axis=1, initial=-FMAX).reshape(P, 1)

    from concourse.dve_spec import eq
    _x3 = Src0 + Src1
    spec3 = Spec(body=eq(_x3, scan(AluOp.MAX, _x3)) * Idx, accum=AluOp.MAX,
                 reference=ref3)

    op3 = make("CRF_BT32", spec3, subdim=False)
    opF = make_segmax()

    # Fused onehot+transpose for the walk: transpose_mode applies a 32x32
    # block transpose to SRC0 only; src1 streams in the output domain
    # (verified on device).  out[32b+c, j] = (in0[32b+j, *] == in1[32b+c, j]).
    from concourse.dve_uop import OpConfig, TransposeMode

    def _blockT(x):
        o = np.empty_like(x)
        for b in range(0, x.shape[0], 32):
            o[b:b + 32, :] = x[b:b + 32, :].T
        return o

    def refT(in0, in1, c0, c1, c2):
        return (_blockT(in0) == in1).astype(np.float32), None

    specT = Spec(body=eq(Src0, Src1), reference=refT)
    opT = make("CRF_OHT2", specT, subdim=False,
               opcfg=OpConfig(transpose_mode=TransposeMode.TRANSPOSE))

    _ops_cache["BT32"] = op3
    _ops_cache["SEGMAX"] = opF
    _ops_cache["OHT2"] = opT
    return op3


def build_nc(Tn):
    assert Tn == T
    register_custom_ops()
    op3 = _ops_cache["BT32"]
    opF = _ops_cache["SEGMAX"]
    opT = _ops_cache["OHT2"]

    NFWD = M          # fwd steps t=1..M
    NBWD = T - 1 - M  # bwd segmax steps t=T-2..M (count 511)
    NU = T - 1 - M    # u slots for t in [M+1, T-1]

    nc = bacc.Bacc("TRN2", target_bir_lowering=False, debug=False,
                   num_devices=NCORES)

    logits = nc.dram_tensor("logits", [PB, T, N], F32, kind="ExternalInput")
    logitsB_d = nc.dram_tensor("logitsB", [PB, T - M, N], F32,
                               kind="ExternalInput")  # biased, slot(t)=t-M
    trepF_d = nc.dram_tensor("trepF", [PB, N * N], F32, kind="ExternalInput")
    trepB_d = nc.dram_tensor("trepB", [PB, N * N], F32, kind="ExternalInput")
    meq_d = nc.dram_tensor("meq", [PB, M + 1], F32, kind="ExternalInput")
    keep_d = nc.dram_tensor("keep", [PB, T], F32, kind="ExternalInput")
    mlt_d = nc.dram_tensor("mlt", [PB, T], F32, kind="ExternalInput")
    mrw_d = nc.dram_tensor("mrw", [PB, M + 1], I8, kind="ExternalInput")
    mrwf_d = nc.dram_tensor("mrwf", [PB, M + 1], F32, kind="ExternalInput")
    mrwc_d = nc.dram_tensor("mrwc", [PB, M + 1], F32, kind="ExternalInput")
    irev_d = nc.dram_tensor("irev", [PB, N], F32, kind="ExternalInput")
    irevp_d = nc.dram_tensor("irevP", [PB, N], F32, kind="ExternalInput")
    trevwF_d = nc.dram_tensor("trevwF", [PB, N], F32, kind="ExternalInput")
    trevwB_d = nc.dram_tensor("trevwB", [PB, N], F32, kind="ExternalInput")
    out_d = nc.dram_tensor("out", [PB, T], I32, kind="ExternalOutput")

    nfch = (M + CH) // CH          # fwd chunks cover t=0..M (33, last partial)
    nbch = (T - M) // CH           # bwd chunks cover t=M..T-1 biased (32)

    with tile.TileContext(nc) as tc:
        with (
            tc.tile_pool(name="consts", bufs=1) as cpool,
            tc.tile_pool(name="states", bufs=1) as spool,
            tc.tile_pool(name="big", bufs=1) as bpool,
            tc.tile_pool(name="lchF", bufs=2) as lpoolF,
            tc.tile_pool(name="lchB", bufs=2) as lpoolB,
            tc.tile_pool(name="small", bufs=1) as mpool,
            tc.tile_pool(name="psum", bufs=1, space="PSUM") as ppool,
        ):
            trepF = cpool.tile([PB, N * N], F32, tag="trepF")
            trepB = cpool.tile([PB, N * N], F32, tag="trepB")
            meq = cpool.tile([PB, M + 1], F32, tag="meq")
            keep = cpool.tile([PB, T], F32, tag="keep")
            mlt = cpool.tile([PB, T], F32, tag="mlt")
            mrw = cpool.tile([PB, M + 1], I8, tag="mrw")
            mrwf = cpool.tile([PB, M + 1], F32, tag="mrwf")
            mrwc = cpool.tile([PB, M + 1], F32, tag="mrwc")
            irev = cpool.tile([PB, N], F32, tag="irev")
            irevP = cpool.tile([PB, N], F32, tag="irevP")
            trevwF = cpool.tile([PB, N], F32, tag="trevwF")
            trevwB = cpool.tile([PB, N], F32, tag="trevwB")

            statesF = spool.tile([PB, (M + 1) * N], F32, tag="statesF")
            uB = spool.tile([PB, NU * N], F32, tag="uB")

            scoresF = bpool.tile([PB, N * N], F32, tag="scoresF")
            scoresB = bpool.tile([PB, N * N], F32, tag="scoresB")
            tags = bpool.tile([PB, T], F32, tag="tags")
            outi = bpool.tile([PB, T], I32, tag="outi")
            accS = bpool.tile([PB, CH * N], F32, tag="accS")
            rsc = bpool.tile([PB, CH * N], F32, tag="rsc")
            rrw = bpool.tile([PB, 64 * N], F32, tag="rrw")

            beta0 = mpool.tile([PB, N], F32, tag="beta0")
            snap = mpool.tile([PB, N], F32, tag="snap")
            lastt = mpool.tile([PB, 1], F32, tag="lastt")
            eqs = mpool.tile([PB, N], F32, tag="eqs")
            red = mpool.tile([PB, N], F32, tag="red")
            m1 = mpool.tile([PB, 1], F32, tag="m1")
            onehot = mpool.tile([PB, N], F32, tag="onehot")
            bigoh = mpool.tile([PB, N], F32, tag="bigoh")
            mbeta = mpool.tile([PB, N], F32, tag="mbeta")
            msc = mpool.tile([PB, N], F32, tag="msc")
            onehotF2 = [mpool.tile([PB, N], F32, name=f"onehotF{x}")
                        for x in range(2)]
            onehotTF2 = [mpool.tile([PB, N], F32, name=f"onehotTF{x}")
                         for x in range(2)]
            redF = mpool.tile([PB, N], F32, tag="redF")
            onehotB2 = [mpool.tile([PB, N], F32, name=f"onehotB{x}")
                        for x in range(2)]
            onehotTB2 = [mpool.tile([PB, N], F32, name=f"onehotTB{x}")
                         for x in range(2)]
            redB = mpool.tile([PB, N], F32, tag="redB")
            tselF2 = [ppool.tile([PB, N], F32, name=f"tselF{x}")
                      for x in range(2)]
            tselB2 = [ppool.tile([PB, N], F32, name=f"tselB{x}")
                      for x in range(2)]

            # ---- first chunks gate the recursion start: issue them first ----
            nc.sync.dma_start(out=trepF[:], in_=trepF_d.ap())
            ltF0 = lpoolF.tile([PB, CH * N], F32, tag="lchF")
            nc.sync.dma_start(
                out=ltF0[:].rearrange("p (t v) -> p t v", v=N),
                in_=logits.ap()[:, 0:CH, :])
            nc.sync.dma_start(out=trepB[:], in_=trepB_d.ap())
            ltB0 = lpoolB.tile([PB, CH * N], F32, tag="lchB")
            nc.sync.dma_start(
                out=ltB0[:].rearrange("p (t v) -> p t v", v=N),
                in_=logitsB_d.ap()[:, T - M - CH:T - M, :])  # t in [1008,1023]
            nc.sync.dma_start(out=keep[:], in_=keep_d.ap())
            nc.sync.dma_start(out=meq[:], in_=meq_d.ap())
            nc.sync.dma_start(out=mlt[:], in_=mlt_d.ap())
            nc.sync.dma_start(out=mrw[:], in_=mrw_d.ap())
            nc.sync.dma_start(out=mrwf[:], in_=mrwf_d.ap())
            nc.sync.dma_start(out=mrwc[:], in_=mrwc_d.ap())
            nc.sync.dma_start(out=irev[:], in_=irev_d.ap())
            nc.sync.dma_start(out=irevP[:], in_=irevp_d.ap())
            nc.sync.dma_start(out=trevwF[:], in_=trevwF_d.ap())
            nc.sync.dma_start(out=trevwB[:], in_=trevwB_d.ap())

            trepF3 = trepF[:].rearrange("p (c v) -> p c v", v=N)
            trepB3 = trepB[:].rearrange("p (c v) -> p c v", v=N)
            scF3 = scoresF[:].rearrange("p (c v) -> p c v", v=N)
            scB3 = scoresB[:].rearrange("p (c v) -> p c v", v=N)
            statesF3 = statesF[:].rearrange("p (t v) -> p t v", v=N)
            uB3 = uB[:].rearrange("p (t v) -> p t v", v=N)
            accS3 = accS[:].rearrange("p (t v) -> p t v", v=N)
            rsc3 = rsc[:].rearrange("p (t v) -> p t v", v=N)

            nc.vector.memset(beta0[:], 0.0)
            nc.gpsimd.memset(accS[:], 0.0)

            # statesF[0] = logits[0]
            nc.vector.tensor_copy(out=statesF[:, 0:N], in_=ltF0[:, 0:N])
            # u[T-1] = beta0*keep[T-1] + logitsB[T-1]  (Pool: 2x tensor_tensor;
            # TensorScalarPtr is rejected on Pool by codegen)
            keepb_last = keep[:, T - 1:T].rearrange("p (a b) -> p a b", a=1) \
                .to_broadcast((PB, N, 1))
            u_last = uB[:, (NU - 1) * N:NU * N].rearrange("p (c o) -> p c o", o=1)
            nc.gpsimd.tensor_tensor(
                out=u_last, in0=beta0[:].rearrange("p (c o) -> p c o", o=1),
                in1=keepb_last, op=OP.mult)
            nc.gpsimd.tensor_tensor(
                out=u_last, in0=u_last,
                in1=ltB0[:, (CH - 1) * N:CH * N].rearrange("p (c o) -> p c o", o=1),
                op=OP.add)

            # ---------------- phase R: interleaved recursions ----------------
            ltF, ltB = ltF0, ltB0
            ltB_next = None
            # u slot for t: t - (M+1); logitsB slot for t: t - M
            for i in range(NFWD):          # i = 0..511
                tf = i + 1                 # fwd computes statesF[tf]
                tb = T - 2 - i             # bwd computes beta_tb (i<NBWD)
                # chunk management
                if tf % CH == 0 and tf < M:
                    ltF = lpoolF.tile([PB, CH * N], F32, tag="lchF")
                    nc.sync.dma_start(
                        out=ltF[:].rearrange("p (t v) -> p t v", v=N),
                        in_=logits.ap()[:, tf:tf + CH, :])
                elif tf == M:
                    ltF = lpoolF.tile([PB, CH * N], F32, tag="lchF")
                    nc.sync.dma_start(
                        out=ltF[:, 0:N].rearrange("p (t v) -> p t v", v=N),
                        in_=logits.ap()[:, M:M + 1, :])
                # bwd chunk k = i//CH + 1 prefetched a full chunk early
                # (issued at i ≡ 0 mod CH, first consumed at i ≡ CH-1)
                if i % CH == 0 and i // CH + 1 < nbch:
                    k = i // CH + 1
                    ltB_next = lpoolB.tile([PB, CH * N], F32, tag="lchB")
                    nc.sync.dma_start(
                        out=ltB_next[:].rearrange("p (t v) -> p t v", v=N),
                        in_=logitsB_d.ap()[:, T - M - CH * (k + 1):
                                           T - M - CH * k, :])
                if (i + 1) % CH == 0:
                    # tb at this i (= T-CH*k-1) already reads the new chunk
                    ltB = ltB_next

                # ---- fwd step ----
                sprev_b = statesF[:, (tf - 1) * N:tf * N] \
                    .rearrange("p (o v) -> p o v", o=1).to_broadcast((PB, N, N))
                nc.vector._custom_dve(opF, out=scF3, in0=sprev_b, in1=trepF3)
                nc.gpsimd.tensor_tensor(
                    out=statesF[:, tf * N:(tf + 1) * N]
                        .rearrange("p (c o) -> p c o", o=1),
                    in0=scF3[:, :, N - 1:N],
                    in1=ltF[:, (tf % CH) * N:(tf % CH + 1) * N]
                        .rearrange("p (c o) -> p c o", o=1),
                    op=OP.add)

                # ---- bwd step ----
                if i < NBWD:
                    usl1 = tb + 1 - (M + 1)
                    u_next_b = uB[:, usl1 * N:(usl1 + 1) * N] \
                        .rearrange("p (o v) -> p o v", o=1) \
                        .to_broadcast((PB, N, N))
                    nc.vector._custom_dve(opF, out=scB3, in0=u_next_b,
                                          in1=trepB3)
                    if tb >= M + 1:
                        usl0 = tb - (M + 1)
                        u_out = uB[:, usl0 * N:(usl0 + 1) * N] \
                            .rearrange("p (c o) -> p c o", o=1)
                        keepb = keep[:, tb:tb + 1] \
                            .rearrange("p (a b) -> p a b", a=1) \
                            .to_broadcast((PB, N, 1))
                        nc.gpsimd.tensor_tensor(
                            out=u_out, in0=scB3[:, :, N - 1:N], in1=keepb,
                            op=OP.mult)
                        nc.gpsimd.tensor_tensor(
                            out=u_out, in0=u_out,
                            in1=ltB[:, (tb % CH) * N:(tb % CH + 1) * N]
                                .rearrange("p (c o) -> p c o", o=1),
                            op=OP.add)

                # ---- snapshot accumulation (Pool), spread in 4-row pieces
                # one chunk behind the recursion to avoid queue bursts ----
                if tf > CH and (tf - 1) % 4 == 0:
                    # piece r of chunk ending at the last CH boundary
                    bnd = ((tf - 1) // CH) * CH     # last boundary < tf
                    r = ((tf - 1) % CH) // 4
                    t0 = bnd - CH + r * 4
                    if t0 + 4 <= bnd:
                        meqb = meq[:, t0:t0 + 4] \
                            .rearrange("p (t o) -> p t o", o=1) \
                            .to_broadcast((PB, 4, N))
                        nc.gpsimd.tensor_tensor(
                            out=rsc3[:, r * 4:r * 4 + 4, :],
                            in0=statesF3[:, t0:t0 + 4, :], in1=meqb,
                            op=OP.mult)
                        nc.gpsimd.tensor_tensor(
                            out=accS3[:, r * 4:r * 4 + 4, :],
                            in0=accS3[:, r * 4:r * 4 + 4, :],
                            in1=rsc3[:, r * 4:r * 4 + 4, :], op=OP.add)

            # remaining snapshot rows: the in-loop pieces covered t in
            # [0, M-CH); add [M-CH, M] (CH+1 rows) here.
            meqb_m = meq[:, M - CH:M + 1].rearrange("p (t o) -> p t o", o=1) \
                .to_broadcast((PB, CH + 1, N))
            rscw = rsc[:].rearrange("p (t v) -> p t v", v=N)
            accw = accS[:].rearrange("p (t v) -> p t v", v=N)
            nc.gpsimd.tensor_tensor(
                out=rscw[:, 0:CH, :], in0=statesF3[:, M - CH:M, :],
                in1=meqb_m[:, 0:CH, :], op=OP.mult)
            nc.gpsimd.tensor_tensor(
                out=accw[:, 0:CH, :], in0=accw[:, 0:CH, :],
                in1=rscw[:, 0:CH, :], op=OP.add)
            nc.gpsimd.tensor_tensor(
                out=rscw[:, 0:1, :], in0=statesF3[:, M:M + 1, :],
                in1=meqb_m[:, CH:CH + 1, :], op=OP.mult)
            nc.gpsimd.tensor_tensor(
                out=accw[:, 0:1, :], in0=accw[:, 0:1, :],
                in1=rscw[:, 0:1, :], op=OP.add)

            # ---------------- phase M: snap -> last_tag -> rewrite -> meet ----
            acc_vt = accS[:].rearrange("p (t v) -> p v t", v=N)
            nc.vector.tensor_reduce(out=snap[:], in_=acc_vt, axis=AX.X,
                                    op=OP.add)
            nc.vector.tensor_reduce(out=m1[:], in_=snap[:], axis=AX.X, op=OP.max)
            nc.vector.tensor_scalar(
                out=eqs[:], in0=snap[:], scalar1=m1[:], scalar2=None,
                op0=OP.is_equal)
            nc.vector.tensor_tensor(out=red[:], in0=eqs[:], in1=irev[:],
                                    op=OP.mult)
            nc.vector.tensor_reduce(out=lastt[:], in_=red[:], axis=AX.X,
                                    op=OP.max)

            # BIG-rewrite of statesF rows t in [len-1, M]; top rows sync via
            # copy_predicated, lower rows async on Pool during the walk.
            RS = M - 64
            RCH = 64
            nc.vector.tensor_scalar(
                out=onehot[:], in0=irev[:], scalar1=lastt[:], scalar2=None,
                op0=OP.is_equal)
            nc.vector.tensor_scalar(
                out=bigoh[:], in0=onehot[:], scalar1=BIG, scalar2=None,
                op0=OP.mult)
            mrwb = mrw[:, RS:M + 1].rearrange("p (t o) -> p t o", o=1) \
                .to_broadcast((PB, M + 1 - RS, N))
            bigohb = bigoh[:].rearrange("p (o v) -> p o v", o=1) \
                .to_broadcast((PB, M + 1 - RS, N))
            nc.vector.copy_predicated(out=statesF3[:, RS:M + 1, :], mask=mrwb,
                                      data=bigohb)
            rrw3 = rrw[:].rearrange("p (t v) -> p t v", v=N)
            bigohc = bigoh[:].rearrange("p (o v) -> p o v", o=1) \
                .to_broadcast((PB, RCH, N))
            for t0 in range(RS - RCH, -1, -RCH):
                stc = statesF3[:, t0:t0 + RCH, :]
                mc = mrwc[:, t0:t0 + RCH] \
                    .rearrange("p (t o) -> p t o", o=1).to_broadcast((PB, RCH, N))
                mf = mrwf[:, t0:t0 + RCH] \
                    .rearrange("p (t o) -> p t o", o=1).to_broadcast((PB, RCH, N))
                nc.gpsimd.tensor_tensor(out=stc, in0=stc, in1=mc, op=OP.mult)
                nc.gpsimd.tensor_tensor(out=rrw3, in0=bigohc, in1=mf,
                                        op=OP.mult)
                nc.gpsimd.tensor_tensor(out=stc, in0=stc, in1=rrw3, op=OP.add)

            # meet: tags[M] = argmax-enc(statesF[M] + beta_M * keep[M])
            nc.vector.tensor_scalar(
                out=mbeta[:].rearrange("p (c o) -> p c o", o=1),
                in0=scB3[:, :, N - 1:N],
                scalar1=keep[:, M:M + 1], scalar2=None, op0=OP.mult)
            nc.vector.tensor_tensor(out=msc[:], in0=statesF[:, M * N:(M + 1) * N],
                                    in1=mbeta[:], op=OP.add)
            nc.vector.tensor_reduce(out=m1[:], in_=msc[:], axis=AX.X, op=OP.max)
            nc.vector.tensor_scalar(
                out=eqs[:], in0=msc[:], scalar1=m1[:], scalar2=None,
                op0=OP.is_equal)
            nc.vector.tensor_tensor(out=red[:], in0=eqs[:], in1=irev[:],
                                    op=OP.mult)
            nc.vector.tensor_reduce(out=tags[:, M:M + 1], in_=red[:],
                                    axis=AX.X, op=OP.max)

            # ---------------- phase W: two interleaved walks ----------------
            for j in range(M):
                tfw = M - 1 - j            # fwd walk target position
                tbw = M + 1 + j            # bwd walk target position
                par = j % 2
                onehotF, onehotTF, tselF = (onehotF2[par], onehotTF2[par],
                                            tselF2[par])
                onehotB, onehotTB, tselB = (onehotB2[par], onehotTB2[par],
                                            tselB2[par])
                # fwd walk step: fused compare + src0 block-transpose
                nc.vector._custom_dve(
                    opT, out=onehotTF[:],
                    in0=tags[:, tfw + 1:tfw + 2].to_broadcast((PB, N)),
                    in1=irevP[:])
                for blk in range(4):
                    nc.tensor.matmul(
                        out=tselF[blk * N:(blk + 1) * N, :],
                        lhsT=onehotTF[blk * N:(blk + 1) * N, :],
                        rhs=trevwF[blk * N:(blk + 1) * N, :],
                        start=True, stop=True,
                        tile_position=(blk * N, blk * N))
                nc.vector._custom_dve(
                    op3, out=redF[:], in0=tselF[:],
                    in1=statesF[:, tfw * N:(tfw + 1) * N][:, ::-1],
                    accum_out=tags[:, tfw:tfw + 1])
                # bwd walk step
                if tbw <= T - 1:
                    nc.vector._custom_dve(
                        opT, out=onehotTB[:],
                        in0=tags[:, tbw - 1:tbw].to_broadcast((PB, N)),
                        in1=irevP[:])
                    for blk in range(4):
                        nc.tensor.matmul(
                            out=tselB[blk * N:(blk + 1) * N, :],
                            lhsT=onehotTB[blk * N:(blk + 1) * N, :],
                            rhs=trevwB[blk * N:(blk + 1) * N, :],
                            start=True, stop=True,
                            tile_position=(blk * N, blk * N))
                    usl = tbw - (M + 1)
                    nc.vector._custom_dve(
                        op3, out=redB[:], in0=tselB[:],
                        in1=uB[:, usl * N:(usl + 1) * N][:, ::-1],
                        accum_out=tags[:, tbw:tbw + 1])

            # ---------------- decode + mask + output ----------------
            nc.vector.tensor_scalar(
                out=tags[:], in0=tags[:], scalar1=-1.0, scalar2=31.0,
                op0=OP.mult, op1=OP.add)
            nc.vector.tensor_tensor(out=outi[:], in0=tags[:], in1=mlt[:],
                                    op=OP.mult)
            nc.sync.dma_start(out=out_d.ap(), in_=outi[:])

    nc.compile()
    return nc


def make_inputs_for_core(logits_shard, lens_shard):
    Tmat = _tmat_holder[0]
    lens = lens_shard.astype(np.int64)[:, None]
    tcolM = np.arange(M + 1)[None, :]
    tcolT = np.arange(T)[None, :]
    meq = (lens == (tcolM + 1)).astype(np.float32)
    keep = (lens - 1 != tcolT).astype(np.float32)
    mlt = (tcolT < lens).astype(np.float32)
    mrw = (tcolM >= (lens - 1)).astype(np.int8)
    irev = (31.0 - np.arange(N, dtype=np.float32))[None, :]
    irevP = np.ascontiguousarray(
        np.broadcast_to((31.0 - (np.arange(PB) % N))[:, None], (PB, N)),
        dtype=np.float32)
    rep = lambda a: np.ascontiguousarray(
        np.broadcast_to(a, (PB, a.shape[1])), dtype=np.float32)
    trepF = np.ascontiguousarray(Tmat.T).reshape(1, N * N)
    trepB = np.ascontiguousarray(Tmat).reshape(1, N * N)
    logitsB = np.ascontiguousarray(
        logits_shard[:, M:, :] - np.float32(BIAS), dtype=np.float32)
    return {
        "logits": np.ascontiguousarray(logits_shard, dtype=np.float32),
        "logitsB": logitsB,
        "trepF": rep(trepF),
        "trepB": rep(trepB),
        "meq": np.ascontiguousarray(meq, dtype=np.float32),
        "keep": np.ascontiguousarray(keep, dtype=np.float32),
        "mlt": np.ascontiguousarray(mlt, dtype=np.float32),
        "mrw": np.ascontiguousarray(mrw, dtype=np.int8),
        "mrwf": np.ascontiguousarray(mrw, dtype=np.float32),
        "mrwc": np.ascontiguousarray(1 - mrw, dtype=np.float32),
        "irev": rep(irev),
        "irevP": irevP,
        "trevwF": np.ascontiguousarray(
            np.tile(Tmat[::-1, :].T, (4, 1)), dtype=np.float32),
        "trevwB": np.ascontiguousarray(
            np.tile(Tmat[:, ::-1], (4, 1)), dtype=np.float32),
    }


_tmat_holder = [None]


def last_exec_time_ns():
    return _last_exec_ns[0]


def kernel(logits, transitions, sequence_lengths, _trace=False):
    logits = np.asarray(logits, dtype=np.float32)
    Tmat = np.asarray(transitions, dtype=np.float32)
    lens = np.asarray(sequence_lengths)
    Bn, Tn, Nn = logits.shape
    assert Nn == N and Bn % NCORES == 0
    _tmat_holder[0] = Tmat

    if Tn not in _nc_cache:
        _nc_cache[Tn] = build_nc(Tn)
    nc = _nc_cache[Tn]

    in_maps = []
    for i in range(NCORES):
        sl = slice(i * PB, (i + 1) * PB)
        in_maps.append(make_inputs_for_core(logits[sl], lens[sl]))

    kw = {}
    if _trace:
        kw = dict(trace=True, trace_cores=[0])
    res = run_bass_kernel_spmd(nc, in_maps, core_ids=list(range(NCORES)), **kw)
    _last_exec_ns[0] = getattr(res, "exec_time_ns", None)

    out = np.concatenate([res.results[i]["out"] for i in range(NCORES)], axis=0)
    return out.astype(np.int32)


# revision 8
# speedup vs baseline: 1.0017x; 1.0013x over previous
"""CRF Viterbi decode (B=1024, T=1024, N=32) on 8 TRN2 NeuronCores — v3.

Meet-in-the-middle: a forward Viterbi recursion over t=[0, M] and a
backward (future-score) relay over t=[1023, M] run as two interleaved
DVE chains, hiding each other's dependency bubbles; the per-step logit
adds run on the Pool engine.  At t=M the two sides meet:
argmax(fwd + bwd) anchors the path, after which two independent
backtrace walks (down from M over fwd states, up from M over stored
backward u-vectors) interleave to hide the DVE->PE->DVE latency.

The backward relay's logits are biased by -BIAS (expected per-step max
gain) so backward scores stay near zero magnitude; the uniform shift
never changes an argmax but keeps f32 rounding noise ~1e-4, far below
typical decision gaps.  The forward side is bit-exact to the reference.
"""
import sys
sys.path.insert(0, "/opt/trn_rl_repo")

import numpy as np

import concourse.bass as bass
import concourse.bacc as bacc
import concourse.mybir as mybir
import concourse.tile as tile
from concourse.bass_utils import run_bass_kernel_spmd

F32 = mybir.dt.float32
I32 = mybir.dt.int32
I8 = mybir.dt.int8
AX = mybir.AxisListType
OP = mybir.AluOpType

B, T, N = 1024, 1024, 32
PB = 128
NCORES = 8
BIG = 1.0e6
M = 512           # meet point
BIAS = 2.1        # backward-relay logit bias
CH = 16           # DMA chunk (time steps)

_ops_cache = {}
_nc_cache = {}
_last_exec_ns = [None]


def register_custom_ops():
    if _ops_cache:
        return _ops_cache["BT32"]
    from concourse.dve_spec import (
        Spec, Src0, Src1, AluOp, lower, Idx, scan, Scan, MaxNeg,
    )
    from concourse.dve_ops import DveOp, OPS, has_src1
    from concourse.dve_uop import DveOpSpec, AluInp
    import concourse.dve_ops as dom

    def make(name, spec, subdim, patch=None, opcfg=None):
        for o in OPS:
            if o.name == name:
                return o
        OPS_len = len(OPS)
        dom._SUB_OPCODE_FOR_NAME[name] = dom._CUSTOM_DVE_ROW_BASE + OPS_len
        assert dom._SUB_OPCODE_FOR_NAME[name] < 0x20
        shas = {}
        for ver in ("v3", "v4"):
            uops = lower(spec, ver=ver)
            if patch is not None:
                patch(uops)
            kw = {} if opcfg is None else {"op": opcfg}
            s = DveOpSpec(name=name, opcode=dom.get_dve_sub_opcode(name),
                          uops=uops, rd1_en=has_src1(spec), **kw)
            shas[ver] = s.sha(ver)
            dom._COMPILE_CACHE[(name, ver)] = s
        op = DveOp(name, spec, subdim=subdim, uops_sha=shas)
        OPS.append(op)
        dom.CUSTOM_DVE_SPECS[name] = spec
        return op

    def make_segmax():
        """Segmented running-max of (Src0 + Src1), reset at each subdim
        (row) boundary."""
        def ref(in0, in1, c0, c1, c2):
            x = (in0 + in1).astype(np.float32)
            r = np.maximum.accumulate(x, axis=-1)
            return r, None

        sc = Scan(AluOp.MAX, Src0 + Src1, _subdim_step=MaxNeg)
        spec = Spec(body=sc, reference=ref)

        def patch(uops):
            assert len(uops) == 3, f"expected [seed, steady, step], got {len(uops)}"
            steady, step = uops[1], uops[2]
            dp = steady.datapath_config[1]
            dp.op = AluOp.MAX
            dp.alu_src0 = AluInp.CURR_ALU_OUT
            dp.alu_src1 = AluInp.PREV_ALU_OUT
            dp = step.datapath_config[1]
            dp.op = AluOp.BYPASS
            dp.alu_src0 = AluInp.PREV_ALU_OUT
            dp.alu_src1 = AluInp.PREV_ALU_OUT

        return make("CRF_SEGMAX", spec, subdim=True, patch=patch)

    FMAX = np.float32(3.4028235e38)

    def ref3(in0, in1, c0, c1, c2):
        P, K = in0.shape
        x = (in0 + in1).astype(np.float32)
        r = np.maximum.accumulate(x, axis=1)
        m = ((x == r).astype(np.float32) * np.arange(K, dtype=np.float32)[None, :])
        return m, m.max(axis=1, initial=-FMAX).reshape(P, 1)

    from concourse.dve_spec import eq
    _x3 = Src0 + Src1
    spec3 = Spec(body=eq(_x3, scan(AluOp.MAX, _x3)) * Idx, accum=AluOp.MAX,
                 reference=ref3)

    op3 = make("CRF_BT32", spec3, subdim=False)
    opF = make_segmax()

    # Fused onehot+transpose for the walk: transpose_mode applies a 32x32
    # block transpose to SRC0 only; src1 streams in the output domain
    # (verified on device).  out[32b+c, j] = (in0[32b+j, *] == in1[32b+c, j]).
    from concourse.dve_uop import OpConfig, TransposeMode

    def _blockT(x):
        o = np.empty_like(x)
        for b in range(0, x.shape[0], 32):
            o[b:b + 32, :] = x[b:b + 32, :].T
        return o

    def refT(in0, in1, c0, c1, c2):
        return (_blockT(in0) == in1).astype(np.float32), None

    specT = Spec(body=eq(Src0, Src1), reference=refT)
    opT = make("CRF_OHT2", specT, subdim=False,
               opcfg=OpConfig(transpose_mode=TransposeMode.TRANSPOSE))

    _ops_cache["BT32"] = op3
    _ops_cache["SEGMAX"] = opF
    _ops_cache["OHT2"] = opT
    return op3


def build_nc(Tn):
    assert Tn == T
    register_custom_ops()
    op3 = _ops_cache["BT32"]
    opF = _ops_cache["SEGMAX"]
    opT = _ops_cache["OHT2"]

    NFWD = M          # fwd steps t=1..M
    NBWD = T - 1 - M  # bwd segmax steps t=T-2..M (count 511)
    NU = T - 1 - M    # u slots for t in [M+1, T-1]

    nc = bacc.Bacc("TRN2", target_bir_lowering=False, debug=False,
                   num_devices=NCORES)

    logits = nc.dram_tensor("logits", [PB, T, N], F32, kind="ExternalInput")
    logitsB_d = nc.dram_tensor("logitsB", [PB, T - M, N], F32,
                               kind="ExternalInput")  # biased, slot(t)=t-M
    trepF_d = nc.dram_tensor("trepF", [PB, N * N], F32, kind="ExternalInput")
    trepB_d = nc.dram_tensor("trepB", [PB, N * N], F32, kind="ExternalInput")
    meq_d = nc.dram_tensor("meq", [PB, M + 1], F32, kind="ExternalInput")
    keep_d = nc.dram_tensor("keep", [PB, T], F32, kind="ExternalInput")
    mlt_d = nc.dram_tensor("mlt", [PB, T], F32, kind="ExternalInput")
    negmlt_d = nc.dram_tensor("negmlt", [PB, T], F32, kind="ExternalInput")
    mrw_d = nc.dram_tensor("mrw", [PB, M + 1], I8, kind="ExternalInput")
    mrwf_d = nc.dram_tensor("mrwf", [PB, M + 1], F32, kind="ExternalInput")
    mrwc_d = nc.dram_tensor("mrwc", [PB, M + 1], F32, kind="ExternalInput")
    irev_d = nc.dram_tensor("irev", [PB, N], F32, kind="ExternalInput")
    irevp_d = nc.dram_tensor("irevP", [PB, N], F32, kind="ExternalInput")
    trevwF_d = nc.dram_tensor("trevwF", [PB, N], F32, kind="ExternalInput")
    trevwB_d = nc.dram_tensor("trevwB", [PB, N], F32, kind="ExternalInput")
    out_d = nc.dram_tensor("out", [PB, T], I32, kind="ExternalOutput")

    nfch = (M + CH) // CH          # fwd chunks cover t=0..M (33, last partial)
    nbch = (T - M) // CH           # bwd chunks cover t=M..T-1 biased (32)

    with tile.TileContext(nc) as tc:
        with (
            tc.tile_pool(name="consts", bufs=1) as cpool,
            tc.tile_pool(name="states", bufs=1) as spool,
            tc.tile_pool(name="big", bufs=1) as bpool,
            tc.tile_pool(name="lchF", bufs=2) as lpoolF,
            tc.tile_pool(name="lchB", bufs=2) as lpoolB,
            tc.tile_pool(name="small", bufs=1) as mpool,
            tc.tile_pool(name="psum", bufs=1, space="PSUM") as ppool,
        ):
            trepF = cpool.tile([PB, N * N], F32, tag="trepF")
            trepB = cpool.tile([PB, N * N], F32, tag="trepB")
            meq = cpool.tile([PB, M + 1], F32, tag="meq")
            keep = cpool.tile([PB, T], F32, tag="keep")
            mlt = cpool.tile([PB, T], F32, tag="mlt")
            negmlt = cpool.tile([PB, T], F32, tag="negmlt")
            mrw = cpool.tile([PB, M + 1], I8, tag="mrw")
            mrwf = cpool.tile([PB, M + 1], F32, tag="mrwf")
            mrwc = cpool.tile([PB, M + 1], F32, tag="mrwc")
            irev = cpool.tile([PB, N], F32, tag="irev")
            irevP = cpool.tile([PB, N], F32, tag="irevP")
            trevwF = cpool.tile([PB, N], F32, tag="trevwF")
            trevwB = cpool.tile([PB, N], F32, tag="trevwB")

            statesF = spool.tile([PB, (M + 1) * N], F32, tag="statesF")
            uB = spool.tile([PB, NU * N], F32, tag="uB")

            scoresF = bpool.tile([PB, N * N], F32, tag="scoresF")
            scoresB = bpool.tile([PB, N * N], F32, tag="scoresB")
            tags = bpool.tile([PB, T], F32, tag="tags")
            outi = bpool.tile([PB, T], I32, tag="outi")
            accS = bpool.tile([PB, CH * N], F32, tag="accS")
            rsc = bpool.tile([PB, CH * N], F32, tag="rsc")
            rrw = bpool.tile([PB, 64 * N], F32, tag="rrw")

            beta0 = mpool.tile([PB, N], F32, tag="beta0")
            snap = mpool.tile([PB, N], F32, tag="snap")
            lastt = mpool.tile([PB, 1], F32, tag="lastt")
            eqs = mpool.tile([PB, N], F32, tag="eqs")
            red = mpool.tile([PB, N], F32, tag="red")
            m1 = mpool.tile([PB, 1], F32, tag="m1")
            onehot = mpool.tile([PB, N], F32, tag="onehot")
            bigoh = mpool.tile([PB, N], F32, tag="bigoh")
            mbeta = mpool.tile([PB, N], F32, tag="mbeta")
            msc = mpool.tile([PB, N], F32, tag="msc")
            onehotF2 = [mpool.tile([PB, N], F32, name=f"onehotF{x}")
                        for x in range(2)]
            onehotTF2 = [mpool.tile([PB, N], F32, name=f"onehotTF{x}")
                         for x in range(2)]
            redF = mpool.tile([PB, N], F32, tag="redF")
            onehotB2 = [mpool.tile([PB, N], F32, name=f"onehotB{x}")
                        for x in range(2)]
            onehotTB2 = [mpool.tile([PB, N], F32, name=f"onehotTB{x}")
                         for x in range(2)]
            redB = mpool.tile([PB, N], F32, tag="redB")
            tselF2 = [ppool.tile([PB, N], F32, name=f"tselF{x}")
                      for x in range(2)]
            tselB2 = [ppool.tile([PB, N], F32, name=f"tselB{x}")
                      for x in range(2)]

            # ---- first chunks gate the recursion start: issue them first ----
            nc.sync.dma_start(out=trepF[:], in_=trepF_d.ap())
            ltF0 = lpoolF.tile([PB, CH * N], F32, tag="lchF")
            nc.sync.dma_start(
                out=ltF0[:].rearrange("p (t v) -> p t v", v=N),
                in_=logits.ap()[:, 0:CH, :])
            nc.sync.dma_start(out=trepB[:], in_=trepB_d.ap())
            ltB0 = lpoolB.tile([PB, CH * N], F32, tag="lchB")
            nc.sync.dma_start(
                out=ltB0[:].rearrange("p (t v) -> p t v", v=N),
                in_=logitsB_d.ap()[:, T - M - CH:T - M, :])  # t in [1008,1023]
            nc.sync.dma_start(out=keep[:], in_=keep_d.ap())
            nc.sync.dma_start(out=meq[:], in_=meq_d.ap())
            nc.sync.dma_start(out=mlt[:], in_=mlt_d.ap())
            nc.sync.dma_start(out=negmlt[:], in_=negmlt_d.ap())
            nc.sync.dma_start(out=mrw[:], in_=mrw_d.ap())
            nc.sync.dma_start(out=mrwf[:], in_=mrwf_d.ap())
            nc.sync.dma_start(out=mrwc[:], in_=mrwc_d.ap())
            nc.sync.dma_start(out=irev[:], in_=irev_d.ap())
            nc.sync.dma_start(out=irevP[:], in_=irevp_d.ap())
            nc.sync.dma_start(out=trevwF[:], in_=trevwF_d.ap())
            nc.sync.dma_start(out=trevwB[:], in_=trevwB_d.ap())

            trepF3 = trepF[:].rearrange("p (c v) -> p c v", v=N)
            trepB3 = trepB[:].rearrange("p (c v) -> p c v", v=N)
            scF3 = scoresF[:].rearrange("p (c v) -> p c v", v=N)
            scB3 = scoresB[:].rearrange("p (c v) -> p c v", v=N)
            statesF3 = statesF[:].rearrange("p (t v) -> p t v", v=N)
            uB3 = uB[:].rearrange("p (t v) -> p t v", v=N)
            accS3 = accS[:].rearrange("p (t v) -> p t v", v=N)
            rsc3 = rsc[:].rearrange("p (t v) -> p t v", v=N)

            nc.vector.memset(beta0[:], 0.0)
            nc.gpsimd.memset(accS[:], 0.0)

            # statesF[0] = logits[0]
            nc.vector.tensor_copy(out=statesF[:, 0:N], in_=ltF0[:, 0:N])
            # u[T-1] = beta0*keep[T-1] + logitsB[T-1]  (Pool: 2x tensor_tensor;
            # TensorScalarPtr is rejected on Pool by codegen)
            keepb_last = keep[:, T - 1:T].rearrange("p (a b) -> p a b", a=1) \
                .to_broadcast((PB, N, 1))
            u_last = uB[:, (NU - 1) * N:NU * N].rearrange("p (c o) -> p c o", o=1)
            nc.gpsimd.tensor_tensor(
                out=u_last, in0=beta0[:].rearrange("p (c o) -> p c o", o=1),
                in1=keepb_last, op=OP.mult)
            nc.gpsimd.tensor_tensor(
                out=u_last, in0=u_last,
                in1=ltB0[:, (CH - 1) * N:CH * N].rearrange("p (c o) -> p c o", o=1),
                op=OP.add)

            # ---------------- phase R: interleaved recursions ----------------
            ltF, ltB = ltF0, ltB0
            ltB_next = None
            # u slot for t: t - (M+1); logitsB slot for t: t - M
            for i in range(NFWD):          # i = 0..511
                tf = i + 1                 # fwd computes statesF[tf]
                tb = T - 2 - i             # bwd computes beta_tb (i<NBWD)
                # chunk management
                if tf % CH == 0 and tf < M:
                    ltF = lpoolF.tile([PB, CH * N], F32, tag="lchF")
                    nc.sync.dma_start(
                        out=ltF[:].rearrange("p (t v) -> p t v", v=N),
                        in_=logits.ap()[:, tf:tf + CH, :])
                elif tf == M:
                    ltF = lpoolF.tile([PB, CH * N], F32, tag="lchF")
                    nc.sync.dma_start(
                        out=ltF[:, 0:N].rearrange("p (t v) -> p t v", v=N),
                        in_=logits.ap()[:, M:M + 1, :])
                # bwd chunk k = i//CH + 1 prefetched a full chunk early
                # (issued at i ≡ 0 mod CH, first consumed at i ≡ CH-1)
                if i % CH == 0 and i // CH + 1 < nbch:
                    k = i // CH + 1
                    ltB_next = lpoolB.tile([PB, CH * N], F32, tag="lchB")
                    nc.sync.dma_start(
                        out=ltB_next[:].rearrange("p (t v) -> p t v", v=N),
                        in_=logitsB_d.ap()[:, T - M - CH * (k + 1):
                                           T - M - CH * k, :])
                if (i + 1) % CH == 0:
                    # tb at this i (= T-CH*k-1) already reads the new chunk
                    ltB = ltB_next

                # ---- fwd step ----
                sprev_b = statesF[:, (tf - 1) * N:tf * N] \
                    .rearrange("p (o v) -> p o v", o=1).to_broadcast((PB, N, N))
                nc.vector._custom_dve(opF, out=scF3, in0=sprev_b, in1=trepF3)
                nc.gpsimd.tensor_tensor(
                    out=statesF[:, tf * N:(tf + 1) * N]
                        .rearrange("p (c o) -> p c o", o=1),
                    in0=scF3[:, :, N - 1:N],
                    in1=ltF[:, (tf % CH) * N:(tf % CH + 1) * N]
                        .rearrange("p (c o) -> p c o", o=1),
                    op=OP.add)

                # ---- bwd step ----
                if i < NBWD:
                    usl1 = tb + 1 - (M + 1)
                    u_next_b = uB[:, usl1 * N:(usl1 + 1) * N] \
                        .rearrange("p (o v) -> p o v", o=1) \
                        .to_broadcast((PB, N, N))
                    nc.vector._custom_dve(opF, out=scB3, in0=u_next_b,
                                          in1=trepB3)
                    if tb >= M + 1:
                        usl0 = tb - (M + 1)
                        u_out = uB[:, usl0 * N:(usl0 + 1) * N] \
                            .rearrange("p (c o) -> p c o", o=1)
                        keepb = keep[:, tb:tb + 1] \
                            .rearrange("p (a b) -> p a b", a=1) \
                            .to_broadcast((PB, N, 1))
                        nc.gpsimd.tensor_tensor(
                            out=u_out, in0=scB3[:, :, N - 1:N], in1=keepb,
                            op=OP.mult)
                        nc.gpsimd.tensor_tensor(
                            out=u_out, in0=u_out,
                            in1=ltB[:, (tb % CH) * N:(tb % CH + 1) * N]
                                .rearrange("p (c o) -> p c o", o=1),
                            op=OP.add)

                # ---- snapshot accumulation (Pool), spread in 4-row pieces
                # one chunk behind the recursion to avoid queue bursts ----
                if tf > CH and (tf - 1) % 4 == 0:
                    # piece r of chunk ending at the last CH boundary
                    bnd = ((tf - 1) // CH) * CH     # last boundary < tf
                    r = ((tf - 1) % CH) // 4
                    t0 = bnd - CH + r * 4
                    if t0 + 4 <= bnd:
                        meqb = meq[:, t0:t0 + 4] \
                            .rearrange("p (t o) -> p t o", o=1) \
                            .to_broadcast((PB, 4, N))
                        nc.gpsimd.tensor_tensor(
                            out=rsc3[:, r * 4:r * 4 + 4, :],
                            in0=statesF3[:, t0:t0 + 4, :], in1=meqb,
                            op=OP.mult)
                        nc.gpsimd.tensor_tensor(
                            out=accS3[:, r * 4:r * 4 + 4, :],
                            in0=accS3[:, r * 4:r * 4 + 4, :],
                            in1=rsc3[:, r * 4:r * 4 + 4, :], op=OP.add)

            # remaining snapshot rows: the in-loop pieces covered t in
            # [0, M-CH); add [M-CH, M] (CH+1 rows) here.
            meqb_m = meq[:, M - CH:M + 1].rearrange("p (t o) -> p t o", o=1) \
                .to_broadcast((PB, CH + 1, N))
            rscw = rsc[:].rearrange("p (t v) -> p t v", v=N)
            accw = accS[:].rearrange("p (t v) -> p t v", v=N)
            nc.gpsimd.tensor_tensor(
                out=rscw[:, 0:CH, :], in0=statesF3[:, M - CH:M, :],
                in1=meqb_m[:, 0:CH, :], op=OP.mult)
            nc.gpsimd.tensor_tensor(
                out=accw[:, 0:CH, :], in0=accw[:, 0:CH, :],
                in1=rscw[:, 0:CH, :], op=OP.add)
            nc.gpsimd.tensor_tensor(
                out=rscw[:, 0:1, :], in0=statesF3[:, M:M + 1, :],
                in1=meqb_m[:, CH:CH + 1, :], op=OP.mult)
            nc.gpsimd.tensor_tensor(
                out=accw[:, 0:1, :], in0=accw[:, 0:1, :],
                in1=rscw[:, 0:1, :], op=OP.add)

            # ---------------- phase M: snap -> last_tag -> rewrite -> meet ----
            acc_vt = accS[:].rearrange("p (t v) -> p v t", v=N)
            nc.vector.tensor_reduce(out=snap[:], in_=acc_vt, axis=AX.X,
                                    op=OP.add)
            nc.vector._custom_dve(
                op3, out=red[:], in0=snap[:, ::-1],
                in1=beta0[:], accum_out=lastt[:])

            # BIG-rewrite of statesF rows t in [len-1, M]; top rows sync via
            # copy_predicated, lower rows async on Pool during the walk.
            RS = M - 64
            RCH = 64
            nc.vector.tensor_scalar(
                out=onehot[:], in0=irev[:], scalar1=lastt[:], scalar2=None,
                op0=OP.is_equal)
            nc.vector.tensor_scalar(
                out=bigoh[:], in0=onehot[:], scalar1=BIG, scalar2=None,
                op0=OP.mult)
            mrwb = mrw[:, RS:M + 1].rearrange("p (t o) -> p t o", o=1) \
                .to_broadcast((PB, M + 1 - RS, N))
            bigohb = bigoh[:].rearrange("p (o v) -> p o v", o=1) \
                .to_broadcast((PB, M + 1 - RS, N))
            nc.vector.copy_predicated(out=statesF3[:, RS:M + 1, :], mask=mrwb,
                                      data=bigohb)
            rrw3 = rrw[:].rearrange("p (t v) -> p t v", v=N)
            bigohc = bigoh[:].rearrange("p (o v) -> p o v", o=1) \
                .to_broadcast((PB, RCH, N))
            for t0 in range(RS - RCH, -1, -RCH):
                stc = statesF3[:, t0:t0 + RCH, :]
                mc = mrwc[:, t0:t0 + RCH] \
                    .rearrange("p (t o) -> p t o", o=1).to_broadcast((PB, RCH, N))
                mf = mrwf[:, t0:t0 + RCH] \
                    .rearrange("p (t o) -> p t o", o=1).to_broadcast((PB, RCH, N))
                nc.gpsimd.tensor_tensor(out=stc, in0=stc, in1=mc, op=OP.mult)
                nc.gpsimd.tensor_tensor(out=rrw3, in0=bigohc, in1=mf,
                                        op=OP.mult)
                nc.gpsimd.tensor_tensor(out=stc, in0=stc, in1=rrw3, op=OP.add)

            # meet: tags[M] = argmax-enc(statesF[M] + beta_M * keep[M])
            nc.vector.tensor_scalar(
                out=mbeta[:].rearrange("p (c o) -> p c o", o=1),
                in0=scB3[:, :, N - 1:N],
                scalar1=keep[:, M:M + 1], scalar2=None, op0=OP.mult)
            # meet argmax in one BT32: x = statesF[M][::-1] + mbeta[::-1]
            nc.vector._custom_dve(
                op3, out=red[:], in0=statesF[:, M * N:(M + 1) * N][:, ::-1],
                in1=mbeta[:, ::-1], accum_out=tags[:, M:M + 1])

            # ---------------- phase W: two interleaved walks ----------------
            for j in range(M):
                tfw = M - 1 - j            # fwd walk target position
                tbw = M + 1 + j            # bwd walk target position
                par = j % 2
                onehotF, onehotTF, tselF = (onehotF2[par], onehotTF2[par],
                                            tselF2[par])
                onehotB, onehotTB, tselB = (onehotB2[par], onehotTB2[par],
                                            tselB2[par])
                # fwd walk step: fused compare + src0 block-transpose
                nc.vector._custom_dve(
                    opT, out=onehotTF[:],
                    in0=tags[:, tfw + 1:tfw + 2].to_broadcast((PB, N)),
                    in1=irevP[:])
                for blk in range(4):
                    nc.tensor.matmul(
                        out=tselF[blk * N:(blk + 1) * N, :],
                        lhsT=onehotTF[blk * N:(blk + 1) * N, :],
                        rhs=trevwF[blk * N:(blk + 1) * N, :],
                        start=True, stop=True,
                        tile_position=(blk * N, blk * N))
                nc.vector._custom_dve(
                    op3, out=redF[:], in0=tselF[:],
                    in1=statesF[:, tfw * N:(tfw + 1) * N][:, ::-1],
                    accum_out=tags[:, tfw:tfw + 1])
                # bwd walk step
                if tbw <= T - 1:
                    nc.vector._custom_dve(
                        opT, out=onehotTB[:],
                        in0=tags[:, tbw - 1:tbw].to_broadcast((PB, N)),
                        in1=irevP[:])
                    for blk in range(4):
                        nc.tensor.matmul(
                            out=tselB[blk * N:(blk + 1) * N, :],
                            lhsT=onehotTB[blk * N:(blk + 1) * N, :],
                            rhs=trevwB[blk * N:(blk + 1) * N, :],
                            start=True, stop=True,
                            tile_position=(blk * N, blk * N))
                    usl = tbw - (M + 1)
                    nc.vector._custom_dve(
                        op3, out=redB[:], in0=tselB[:],
                        in1=uB[:, usl * N:(usl + 1) * N][:, ::-1],
                        accum_out=tags[:, tbw:tbw + 1])

            # ---------------- decode + mask + output ----------------
            # outi = (enc - 31) * (-mlt) == (31 - enc) * mlt, one 2x-rate op
            nc.vector.scalar_tensor_tensor(
                out=outi[:], in0=tags[:], scalar=31.0, in1=negmlt[:],
                op0=OP.subtract, op1=OP.mult)
            nc.sync.dma_start(out=out_d.ap(), in_=outi[:])

    nc.compile()
    return nc


def make_inputs_for_core(logits_shard, lens_shard):
    Tmat = _tmat_holder[0]
    lens = lens_shard.astype(np.int64)[:, None]
    tcolM = np.arange(M + 1)[None, :]
    tcolT = np.arange(T)[None, :]
    meq = (lens == (tcolM + 1)).astype(np.float32)
    keep = (lens - 1 != tcolT).astype(np.float32)
    mlt = (tcolT < lens).astype(np.float32)
    mrw = (tcolM >= (lens - 1)).astype(np.int8)
    irev = (31.0 - np.arange(N, dtype=np.float32))[None, :]
    irevP = np.ascontiguousarray(
        np.broadcast_to((31.0 - (np.arange(PB) % N))[:, None], (PB, N)),
        dtype=np.float32)
    rep = lambda a: np.ascontiguousarray(
        np.broadcast_to(a, (PB, a.shape[1])), dtype=np.float32)
    trepF = np.ascontiguousarray(Tmat.T).reshape(1, N * N)
    trepB = np.ascontiguousarray(Tmat).reshape(1, N * N)
    logitsB = np.ascontiguousarray(
        logits_shard[:, M:, :] - np.float32(BIAS), dtype=np.float32)
    return {
        "logits": np.ascontiguousarray(logits_shard, dtype=np.float32),
        "logitsB": logitsB,
        "trepF": rep(trepF),
        "trepB": rep(trepB),
        "meq": np.ascontiguousarray(meq, dtype=np.float32),
        "keep": np.ascontiguousarray(keep, dtype=np.float32),
        "mlt": np.ascontiguousarray(mlt, dtype=np.float32),
        "negmlt": np.ascontiguousarray(-mlt, dtype=np.float32),
        "mrw": np.ascontiguousarray(mrw, dtype=np.int8),
        "mrwf": np.ascontiguousarray(mrw, dtype=np.float32),
        "mrwc": np.ascontiguousarray(1 - mrw, dtype=np.float32),
        "irev": rep(irev),
        "irevP": irevP,
        "trevwF": np.ascontiguousarray(
            np.tile(Tmat[::-1, :].T, (4, 1)), dtype=np.float32),
        "trevwB": np.ascontiguousarray(
            np.tile(Tmat[:, ::-1], (4, 1)), dtype=np.float32),
    }


_tmat_holder = [None]


def last_exec_time_ns():
    return _last_exec_ns[0]


def kernel(logits, transitions, sequence_lengths, _trace=False):
    logits = np.asarray(logits, dtype=np.float32)
    Tmat = np.asarray(transitions, dtype=np.float32)
    lens = np.asarray(sequence_lengths)
    Bn, Tn, Nn = logits.shape
    assert Nn == N and Bn % NCORES == 0
    _tmat_holder[0] = Tmat

    if Tn not in _nc_cache:
        _nc_cache[Tn] = build_nc(Tn)
    nc = _nc_cache[Tn]

    in_maps = []
    for i in range(NCORES):
        sl = slice(i * PB, (i + 1) * PB)
        in_maps.append(make_inputs_for_core(logits[sl], lens[sl]))

    kw = {}
    if _trace:
        kw = dict(trace=True, trace_cores=[0])
    res = run_bass_kernel_spmd(nc, in_maps, core_ids=list(range(NCORES)), **kw)
    _last_exec_ns[0] = getattr(res, "exec_time_ns", None)

    out = np.concatenate([res.results[i]["out"] for i in range(NCORES)], axis=0)
    return out.astype(np.int32)


# revision 9
# speedup vs baseline: 1.0023x; 1.0005x over previous
"""CRF Viterbi decode (B=1024, T=1024, N=32) on 8 TRN2 NeuronCores — v3.

Meet-in-the-middle: a forward Viterbi recursion over t=[0, M] and a
backward (future-score) relay over t=[1023, M] run as two interleaved
DVE chains, hiding each other's dependency bubbles; the per-step logit
adds run on the Pool engine.  At t=M the two sides meet:
argmax(fwd + bwd) anchors the path, after which two independent
backtrace walks (down from M over fwd states, up from M over stored
backward u-vectors) interleave to hide the DVE->PE->DVE latency.

The backward relay's logits are biased by -BIAS (expected per-step max
gain) so backward scores stay near zero magnitude; the uniform shift
never changes an argmax but keeps f32 rounding noise ~1e-4, far below
typical decision gaps.  The forward side is bit-exact to the reference.
"""
import sys
sys.path.insert(0, "/opt/trn_rl_repo")

import numpy as np

import concourse.bass as bass
import concourse.bacc as bacc
import concourse.mybir as mybir
import concourse.tile as tile
from concourse.bass_utils import run_bass_kernel_spmd

F32 = mybir.dt.float32
I32 = mybir.dt.int32
I8 = mybir.dt.int8
AX = mybir.AxisListType
OP = mybir.AluOpType

B, T, N = 1024, 1024, 32
PB = 128
NCORES = 8
BIG = 1.0e6
M = 512           # meet point
BIAS = 2.1        # backward-relay logit bias
CH = 16           # DMA chunk (time steps)

_ops_cache = {}
_nc_cache = {}
_last_exec_ns = [None]


def register_custom_ops():
    if _ops_cache:
        return _ops_cache["BT32"]
    from concourse.dve_spec import (
        Spec, Src0, Src1, AluOp, lower, Idx, scan, Scan, MaxNeg,
    )
    from concourse.dve_ops import DveOp, OPS, has_src1
    from concourse.dve_uop import DveOpSpec, AluInp
    import concourse.dve_ops as dom

    def make(name, spec, subdim, patch=None, opcfg=None):
        for o in OPS:
            if o.name == name:
                return o
        OPS_len = len(OPS)
        dom._SUB_OPCODE_FOR_NAME[name] = dom._CUSTOM_DVE_ROW_BASE + OPS_len
        assert dom._SUB_OPCODE_FOR_NAME[name] < 0x20
        shas = {}
        for ver in ("v3", "v4"):
            uops = lower(spec, ver=ver)
            if patch is not None:
                patch(uops)
            kw = {} if opcfg is None else {"op": opcfg}
            s = DveOpSpec(name=name, opcode=dom.get_dve_sub_opcode(name),
                          uops=uops, rd1_en=has_src1(spec), **kw)
            shas[ver] = s.sha(ver)
            dom._COMPILE_CACHE[(name, ver)] = s
        op = DveOp(name, spec, subdim=subdim, uops_sha=shas)
        OPS.append(op)
        dom.CUSTOM_DVE_SPECS[name] = spec
        return op

    def make_segmax():
        """Segmented running-max of (Src0 + Src1), reset at each subdim
        (row) boundary."""
        def ref(in0, in1, c0, c1, c2):
            x = (in0 + in1).astype(np.float32)
            r = np.maximum.accumulate(x, axis=-1)
            return r, None

        sc = Scan(AluOp.MAX, Src0 + Src1, _subdim_step=MaxNeg)
        spec = Spec(body=sc, reference=ref)

        def patch(uops):
            assert len(uops) == 3, f"expected [seed, steady, step], got {len(uops)}"
            steady, step = uops[1], uops[2]
            dp = steady.datapath_config[1]
            dp.op = AluOp.MAX
            dp.alu_src0 = AluInp.CURR_ALU_OUT
            dp.alu_src1 = AluInp.PREV_ALU_OUT
            dp = step.datapath_config[1]
            dp.op = AluOp.BYPASS
            dp.alu_src0 = AluInp.PREV_ALU_OUT
            dp.alu_src1 = AluInp.PREV_ALU_OUT

        return make("CRF_SEGMAX", spec, subdim=True, patch=patch)

    FMAX = np.float32(3.4028235e38)

    def ref3(in0, in1, c0, c1, c2):
        P, K = in0.shape
        x = (in0 + in1).astype(np.float32)
        r = np.maximum.accumulate(x, axis=1)
        m = ((x == r).astype(np.float32) * np.arange(K, dtype=np.float32)[None, :])
        return m, m.max(axis=1, initial=-FMAX).reshape(P, 1)

    from concourse.dve_spec import eq
    _x3 = Src0 + Src1
    spec3 = Spec(body=eq(_x3, scan(AluOp.MAX, _x3)) * Idx, accum=AluOp.MAX,
                 reference=ref3)

    op3 = make("CRF_BT32", spec3, subdim=False)
    opF = make_segmax()

    # Fused onehot+transpose for the walk: transpose_mode applies a 32x32
    # block transpose to SRC0 only; src1 streams in the output domain
    # (verified on device).  out[32b+c, j] = (in0[32b+j, *] == in1[32b+c, j]).
    from concourse.dve_uop import OpConfig, TransposeMode

    def _blockT(x):
        o = np.empty_like(x)
        for b in range(0, x.shape[0], 32):
            o[b:b + 32, :] = x[b:b + 32, :].T
        return o

    def refT(in0, in1, c0, c1, c2):
        return (_blockT(in0) == in1).astype(np.float32), None

    specT = Spec(body=eq(Src0, Src1), reference=refT)
    opT = make("CRF_OHT2", specT, subdim=False,
               opcfg=OpConfig(transpose_mode=TransposeMode.TRANSPOSE))

    _ops_cache["BT32"] = op3
    _ops_cache["SEGMAX"] = opF
    _ops_cache["OHT2"] = opT
    return op3


def build_nc(Tn):
    assert Tn == T
    register_custom_ops()
    op3 = _ops_cache["BT32"]
    opF = _ops_cache["SEGMAX"]
    opT = _ops_cache["OHT2"]

    NFWD = M          # fwd steps t=1..M
    NBWD = T - 1 - M  # bwd segmax steps t=T-2..M (count 511)
    NU = T - 1 - M    # u slots for t in [M+1, T-1]

    nc = bacc.Bacc("TRN2", target_bir_lowering=False, debug=False,
                   num_devices=NCORES)

    logits = nc.dram_tensor("logits", [PB, T, N], F32, kind="ExternalInput")
    logitsB_d = nc.dram_tensor("logitsB", [PB, T - M, N], F32,
                               kind="ExternalInput")  # biased, slot(t)=t-M
    trepF_d = nc.dram_tensor("trepF", [PB, N * N], F32, kind="ExternalInput")
    trepB_d = nc.dram_tensor("trepB", [PB, N * N], F32, kind="ExternalInput")
    meq_d = nc.dram_tensor("meq", [PB, M + 1], F32, kind="ExternalInput")
    keep_d = nc.dram_tensor("keep", [PB, T], F32, kind="ExternalInput")
    mlt_d = nc.dram_tensor("mlt", [PB, T], F32, kind="ExternalInput")
    negmlt_d = nc.dram_tensor("negmlt", [PB, T], F32, kind="ExternalInput")
    mrw_d = nc.dram_tensor("mrw", [PB, M + 1], I8, kind="ExternalInput")
    mrwf_d = nc.dram_tensor("mrwf", [PB, M + 1], F32, kind="ExternalInput")
    mrwc_d = nc.dram_tensor("mrwc", [PB, M + 1], F32, kind="ExternalInput")
    irev_d = nc.dram_tensor("irev", [PB, N], F32, kind="ExternalInput")
    irevp_d = nc.dram_tensor("irevP", [PB, N], F32, kind="ExternalInput")
    trevwF_d = nc.dram_tensor("trevwF", [PB, N], F32, kind="ExternalInput")
    trevwB_d = nc.dram_tensor("trevwB", [PB, N], F32, kind="ExternalInput")
    out_d = nc.dram_tensor("out", [PB, T], I32, kind="ExternalOutput")

    nfch = (M + CH) // CH          # fwd chunks cover t=0..M (33, last partial)
    nbch = (T - M) // CH           # bwd chunks cover t=M..T-1 biased (32)

    with tile.TileContext(nc) as tc:
        with (
            tc.tile_pool(name="consts", bufs=1) as cpool,
            tc.tile_pool(name="states", bufs=1) as spool,
            tc.tile_pool(name="big", bufs=1) as bpool,
            tc.tile_pool(name="lchF", bufs=2) as lpoolF,
            tc.tile_pool(name="lchB", bufs=2) as lpoolB,
            tc.tile_pool(name="small", bufs=1) as mpool,
            tc.tile_pool(name="psum", bufs=1, space="PSUM") as ppool,
        ):
            trepF = cpool.tile([PB, N * N], F32, tag="trepF")
            trepB = cpool.tile([PB, N * N], F32, tag="trepB")
            meq = cpool.tile([PB, M + 1], F32, tag="meq")
            keep = cpool.tile([PB, T], F32, tag="keep")
            mlt = cpool.tile([PB, T], F32, tag="mlt")
            negmlt = cpool.tile([PB, T], F32, tag="negmlt")
            mrw = cpool.tile([PB, M + 1], I8, tag="mrw")
            mrwf = cpool.tile([PB, M + 1], F32, tag="mrwf")
            mrwc = cpool.tile([PB, M + 1], F32, tag="mrwc")
            irev = cpool.tile([PB, N], F32, tag="irev")
            irevP = cpool.tile([PB, N], F32, tag="irevP")
            trevwF = cpool.tile([PB, N], F32, tag="trevwF")
            trevwB = cpool.tile([PB, N], F32, tag="trevwB")

            statesF = spool.tile([PB, (M + 1) * N], F32, tag="statesF")
            uB = spool.tile([PB, NU * N], F32, tag="uB")

            scoresF = bpool.tile([PB, N * N], F32, tag="scoresF")
            scoresB = bpool.tile([PB, N * N], F32, tag="scoresB")
            tags = bpool.tile([PB, T], F32, tag="tags")
            outi = bpool.tile([PB, T], I32, tag="outi")
            accS = bpool.tile([PB, CH * N], F32, tag="accS")
            rsc = bpool.tile([PB, CH * N], F32, tag="rsc")
            rrw = bpool.tile([PB, 64 * N], F32, tag="rrw")

            beta0 = mpool.tile([PB, N], F32, tag="beta0")
            snap = mpool.tile([PB, N], F32, tag="snap")
            lastt = mpool.tile([PB, 1], F32, tag="lastt")
            eqs = mpool.tile([PB, N], F32, tag="eqs")
            red = mpool.tile([PB, N], F32, tag="red")
            m1 = mpool.tile([PB, 1], F32, tag="m1")
            onehot = mpool.tile([PB, N], F32, tag="onehot")
            bigoh = mpool.tile([PB, N], F32, tag="bigoh")
            mbeta = mpool.tile([PB, N], F32, tag="mbeta")
            msc = mpool.tile([PB, N], F32, tag="msc")
            onehotF2 = [mpool.tile([PB, N], F32, name=f"onehotF{x}")
                        for x in range(2)]
            onehotTF2 = [mpool.tile([PB, N], F32, name=f"onehotTF{x}")
                         for x in range(2)]
            redF = mpool.tile([PB, N], F32, tag="redF")
            onehotB2 = [mpool.tile([PB, N], F32, name=f"onehotB{x}")
                        for x in range(2)]
            onehotTB2 = [mpool.tile([PB, N], F32, name=f"onehotTB{x}")
                         for x in range(2)]
            redB = mpool.tile([PB, N], F32, tag="redB")
            tselF2 = [ppool.tile([PB, N], F32, name=f"tselF{x}")
                      for x in range(2)]
            tselB2 = [ppool.tile([PB, N], F32, name=f"tselB{x}")
                      for x in range(2)]

            # ---- first chunks gate the recursion start: issue them first ----
            nc.sync.dma_start(out=trepF[:], in_=trepF_d.ap())
            ltF0 = lpoolF.tile([PB, CH * N], F32, tag="lchF")
            nc.sync.dma_start(
                out=ltF0[:].rearrange("p (t v) -> p t v", v=N),
                in_=logits.ap()[:, 0:CH, :])
            nc.sync.dma_start(out=trepB[:], in_=trepB_d.ap())
            ltB0 = lpoolB.tile([PB, CH * N], F32, tag="lchB")
            nc.sync.dma_start(
                out=ltB0[:].rearrange("p (t v) -> p t v", v=N),
                in_=logitsB_d.ap()[:, T - M - CH:T - M, :])  # t in [1008,1023]
            nc.sync.dma_start(out=keep[:], in_=keep_d.ap())
            nc.sync.dma_start(out=meq[:], in_=meq_d.ap())
            nc.sync.dma_start(out=mlt[:], in_=mlt_d.ap())
            nc.sync.dma_start(out=negmlt[:], in_=negmlt_d.ap())
            nc.sync.dma_start(out=mrw[:], in_=mrw_d.ap())
            nc.sync.dma_start(out=mrwf[:], in_=mrwf_d.ap())
            nc.sync.dma_start(out=mrwc[:], in_=mrwc_d.ap())
            nc.sync.dma_start(out=irev[:], in_=irev_d.ap())
            nc.sync.dma_start(out=irevP[:], in_=irevp_d.ap())
            nc.sync.dma_start(out=trevwF[:], in_=trevwF_d.ap())
            nc.sync.dma_start(out=trevwB[:], in_=trevwB_d.ap())

            trepF3 = trepF[:].rearrange("p (c v) -> p c v", v=N)
            trepB3 = trepB[:].rearrange("p (c v) -> p c v", v=N)
            scF3 = scoresF[:].rearrange("p (c v) -> p c v", v=N)
            scB3 = scoresB[:].rearrange("p (c v) -> p c v", v=N)
            statesF3 = statesF[:].rearrange("p (t v) -> p t v", v=N)
            uB3 = uB[:].rearrange("p (t v) -> p t v", v=N)
            accS3 = accS[:].rearrange("p (t v) -> p t v", v=N)
            rsc3 = rsc[:].rearrange("p (t v) -> p t v", v=N)

            nc.vector.memset(beta0[:], 0.0)
            nc.gpsimd.memset(accS[:], 0.0)

            # statesF[0] = logits[0]
            nc.vector.tensor_copy(out=statesF[:, 0:N], in_=ltF0[:, 0:N])
            # u[T-1] = beta0*keep[T-1] + logitsB[T-1]  (Pool: 2x tensor_tensor;
            # TensorScalarPtr is rejected on Pool by codegen)
            keepb_last = keep[:, T - 1:T].rearrange("p (a b) -> p a b", a=1) \
                .to_broadcast((PB, N, 1))
            u_last = uB[:, (NU - 1) * N:NU * N].rearrange("p (c o) -> p c o", o=1)
            nc.gpsimd.tensor_tensor(
                out=u_last, in0=beta0[:].rearrange("p (c o) -> p c o", o=1),
                in1=keepb_last, op=OP.mult)
            nc.gpsimd.tensor_tensor(
                out=u_last, in0=u_last,
                in1=ltB0[:, (CH - 1) * N:CH * N].rearrange("p (c o) -> p c o", o=1),
                op=OP.add)

            # ---------------- phase R: interleaved recursions ----------------
            ltF, ltB = ltF0, ltB0
            ltB_next = None
            # u slot for t: t - (M+1); logitsB slot for t: t - M
            for i in range(NFWD):          # i = 0..511
                tf = i + 1                 # fwd computes statesF[tf]
                tb = T - 2 - i             # bwd computes beta_tb (i<NBWD)
                # chunk management
                if tf % CH == 0 and tf < M:
                    ltF = lpoolF.tile([PB, CH * N], F32, tag="lchF")
                    nc.sync.dma_start(
                        out=ltF[:].rearrange("p (t v) -> p t v", v=N),
                        in_=logits.ap()[:, tf:tf + CH, :])
                elif tf == M:
                    ltF = lpoolF.tile([PB, CH * N], F32, tag="lchF")
                    nc.sync.dma_start(
                        out=ltF[:, 0:N].rearrange("p (t v) -> p t v", v=N),
                        in_=logits.ap()[:, M:M + 1, :])
                # bwd chunk k = i//CH + 1 prefetched a full chunk early
                # (issued at i ≡ 0 mod CH, first consumed at i ≡ CH-1)
                if i % CH == 0 and i // CH + 1 < nbch:
                    k = i // CH + 1
                    ltB_next = lpoolB.tile([PB, CH * N], F32, tag="lchB")
                    nc.sync.dma_start(
                        out=ltB_next[:].rearrange("p (t v) -> p t v", v=N),
                        in_=logitsB_d.ap()[:, T - M - CH * (k + 1):
                                           T - M - CH * k, :])
                if (i + 1) % CH == 0:
                    # tb at this i (= T-CH*k-1) already reads the new chunk
                    ltB = ltB_next

                # ---- fwd step ----
                sprev_b = statesF[:, (tf - 1) * N:tf * N] \
                    .rearrange("p (o v) -> p o v", o=1).to_broadcast((PB, N, N))
                nc.vector._custom_dve(opF, out=scF3, in0=sprev_b, in1=trepF3)
                nc.gpsimd.tensor_tensor(
                    out=statesF[:, tf * N:(tf + 1) * N]
                        .rearrange("p (c o) -> p c o", o=1),
                    in0=scF3[:, :, N - 1:N],
                    in1=ltF[:, (tf % CH) * N:(tf % CH + 1) * N]
                        .rearrange("p (c o) -> p c o", o=1),
                    op=OP.add)

                # ---- bwd step ----
                if i < NBWD:
                    usl1 = tb + 1 - (M + 1)
                    u_next_b = uB[:, usl1 * N:(usl1 + 1) * N] \
                        .rearrange("p (o v) -> p o v", o=1) \
                        .to_broadcast((PB, N, N))
                    nc.vector._custom_dve(opF, out=scB3, in0=u_next_b,
                                          in1=trepB3)
                    if tb >= M + 1:
                        usl0 = tb - (M + 1)
                        u_out = uB[:, usl0 * N:(usl0 + 1) * N] \
                            .rearrange("p (c o) -> p c o", o=1)
                        keepb = keep[:, tb:tb + 1] \
                            .rearrange("p (a b) -> p a b", a=1) \
                            .to_broadcast((PB, N, 1))
                        nc.gpsimd.tensor_tensor(
                            out=u_out, in0=scB3[:, :, N - 1:N], in1=keepb,
                            op=OP.mult)
                        nc.gpsimd.tensor_tensor(
                            out=u_out, in0=u_out,
                            in1=ltB[:, (tb % CH) * N:(tb % CH + 1) * N]
                                .rearrange("p (c o) -> p c o", o=1),
                            op=OP.add)

                # late rows [M-CH, M-4): accumulate as soon as written so
                # only a 4+1-row tail gates the meet
                if tf >= M - CH + 5 and (tf - (M - CH + 5)) % 4 == 0 \
                        and tf <= M - 3:
                    t0l = tf - 5
                    rl = (t0l - (M - CH)) // 4
                    meqbl = meq[:, t0l:t0l + 4] \
                        .rearrange("p (t o) -> p t o", o=1) \
                        .to_broadcast((PB, 4, N))
                    nc.gpsimd.tensor_tensor(
                        out=rsc3[:, rl * 4:rl * 4 + 4, :],
                        in0=statesF3[:, t0l:t0l + 4, :], in1=meqbl,
                        op=OP.mult)
                    nc.gpsimd.tensor_tensor(
                        out=accS3[:, rl * 4:rl * 4 + 4, :],
                        in0=accS3[:, rl * 4:rl * 4 + 4, :],
                        in1=rsc3[:, rl * 4:rl * 4 + 4, :], op=OP.add)
                # ---- snapshot accumulation (Pool), spread in 4-row pieces
                # one chunk behind the recursion to avoid queue bursts ----
                if tf > CH and (tf - 1) % 4 == 0:
                    # piece r of chunk ending at the last CH boundary
                    bnd = ((tf - 1) // CH) * CH     # last boundary < tf
                    r = ((tf - 1) % CH) // 4
                    t0 = bnd - CH + r * 4
                    if t0 + 4 <= bnd:
                        meqb = meq[:, t0:t0 + 4] \
                            .rearrange("p (t o) -> p t o", o=1) \
                            .to_broadcast((PB, 4, N))
                        nc.gpsimd.tensor_tensor(
                            out=rsc3[:, r * 4:r * 4 + 4, :],
                            in0=statesF3[:, t0:t0 + 4, :], in1=meqb,
                            op=OP.mult)
                        nc.gpsimd.tensor_tensor(
                            out=accS3[:, r * 4:r * 4 + 4, :],
                            in0=accS3[:, r * 4:r * 4 + 4, :],
                            in1=rsc3[:, r * 4:r * 4 + 4, :], op=OP.add)

            # remaining snapshot rows: in-loop pieces covered [0, M-4);
            # add [M-4, M] (5 rows) here.
            meqb_m = meq[:, M - 4:M + 1].rearrange("p (t o) -> p t o", o=1) \
                .to_broadcast((PB, 5, N))
            rscw = rsc[:].rearrange("p (t v) -> p t v", v=N)
            accw = accS[:].rearrange("p (t v) -> p t v", v=N)
            nc.gpsimd.tensor_tensor(
                out=rscw[:, 0:4, :], in0=statesF3[:, M - 4:M, :],
                in1=meqb_m[:, 0:4, :], op=OP.mult)
            nc.gpsimd.tensor_tensor(
                out=accw[:, 0:4, :], in0=accw[:, 0:4, :],
                in1=rscw[:, 0:4, :], op=OP.add)
            nc.gpsimd.tensor_tensor(
                out=rscw[:, 0:1, :], in0=statesF3[:, M:M + 1, :],
                in1=meqb_m[:, 4:5, :], op=OP.mult)
            nc.gpsimd.tensor_tensor(
                out=accw[:, 0:1, :], in0=accw[:, 0:1, :],
                in1=rscw[:, 0:1, :], op=OP.add)

            # ---------------- phase M: snap -> last_tag -> rewrite -> meet ----
            acc_vt = accS[:].rearrange("p (t v) -> p v t", v=N)
            nc.vector.tensor_reduce(out=snap[:], in_=acc_vt, axis=AX.X,
                                    op=OP.add)
            nc.vector._custom_dve(
                op3, out=red[:], in0=snap[:, ::-1],
                in1=beta0[:], accum_out=lastt[:])

            # BIG-rewrite of statesF rows t in [len-1, M]; top rows sync via
            # copy_predicated, lower rows async on Pool during the walk.
            RS = M - 64
            RCH = 64
            nc.vector.tensor_scalar(
                out=onehot[:], in0=irev[:], scalar1=lastt[:], scalar2=None,
                op0=OP.is_equal)
            nc.vector.tensor_scalar(
                out=bigoh[:], in0=onehot[:], scalar1=BIG, scalar2=None,
                op0=OP.mult)
            # rows [M-16, M] first: they gate the meet and the walk's start;
            # the rest follows while the meet chain runs
            RS2 = M - 16
            mrwb2 = mrw[:, RS2:M + 1].rearrange("p (t o) -> p t o", o=1) \
                .to_broadcast((PB, M + 1 - RS2, N))
            bigohb2 = bigoh[:].rearrange("p (o v) -> p o v", o=1) \
                .to_broadcast((PB, M + 1 - RS2, N))
            nc.vector.copy_predicated(out=statesF3[:, RS2:M + 1, :],
                                      mask=mrwb2, data=bigohb2)
            # meet: mbeta + anchor argmax (needs only row M)
            nc.vector.tensor_scalar(
                out=mbeta[:].rearrange("p (c o) -> p c o", o=1),
                in0=scB3[:, :, N - 1:N],
                scalar1=keep[:, M:M + 1], scalar2=None, op0=OP.mult)
            nc.vector._custom_dve(
                op3, out=red[:], in0=statesF[:, M * N:(M + 1) * N][:, ::-1],
                in1=mbeta[:, ::-1], accum_out=tags[:, M:M + 1])
            mrwb = mrw[:, RS:RS2].rearrange("p (t o) -> p t o", o=1) \
                .to_broadcast((PB, RS2 - RS, N))
            bigohb = bigoh[:].rearrange("p (o v) -> p o v", o=1) \
                .to_broadcast((PB, RS2 - RS, N))
            nc.vector.copy_predicated(out=statesF3[:, RS:RS2, :], mask=mrwb,
                                      data=bigohb)
            rrw3 = rrw[:].rearrange("p (t v) -> p t v", v=N)
            bigohc = bigoh[:].rearrange("p (o v) -> p o v", o=1) \
                .to_broadcast((PB, RCH, N))
            for t0 in range(RS - RCH, -1, -RCH):
                stc = statesF3[:, t0:t0 + RCH, :]
                mc = mrwc[:, t0:t0 + RCH] \
                    .rearrange("p (t o) -> p t o", o=1).to_broadcast((PB, RCH, N))
                mf = mrwf[:, t0:t0 + RCH] \
                    .rearrange("p (t o) -> p t o", o=1).to_broadcast((PB, RCH, N))
                nc.gpsimd.tensor_tensor(out=stc, in0=stc, in1=mc, op=OP.mult)
                nc.gpsimd.tensor_tensor(out=rrw3, in0=bigohc, in1=mf,
                                        op=OP.mult)
                nc.gpsimd.tensor_tensor(out=stc, in0=stc, in1=rrw3, op=OP.add)

            # ---------------- phase W: two interleaved walks ----------------
            for j in range(M):
                tfw = M - 1 - j            # fwd walk target position
                tbw = M + 1 + j            # bwd walk target position
                par = j % 2
                onehotF, onehotTF, tselF = (onehotF2[par], onehotTF2[par],
                                            tselF2[par])
                onehotB, onehotTB, tselB = (onehotB2[par], onehotTB2[par],
                                            tselB2[par])
                # fwd walk step: fused compare + src0 block-transpose
                nc.vector._custom_dve(
                    opT, out=onehotTF[:],
                    in0=tags[:, tfw + 1:tfw + 2].to_broadcast((PB, N)),
                    in1=irevP[:])
                for blk in range(4):
                    nc.tensor.matmul(
                        out=tselF[blk * N:(blk + 1) * N, :],
                        lhsT=onehotTF[blk * N:(blk + 1) * N, :],
                        rhs=trevwF[blk * N:(blk + 1) * N, :],
                        start=True, stop=True,
                        tile_position=(blk * N, blk * N))
                nc.vector._custom_dve(
                    op3, out=redF[:], in0=tselF[:],
                    in1=statesF[:, tfw * N:(tfw + 1) * N][:, ::-1],
                    accum_out=tags[:, tfw:tfw + 1])
                # bwd walk step
                if tbw <= T - 1:
                    nc.vector._custom_dve(
                        opT, out=onehotTB[:],
                        in0=tags[:, tbw - 1:tbw].to_broadcast((PB, N)),
                        in1=irevP[:])
                    for blk in range(4):
                        nc.tensor.matmul(
                            out=tselB[blk * N:(blk + 1) * N, :],
                            lhsT=onehotTB[blk * N:(blk + 1) * N, :],
                            rhs=trevwB[blk * N:(blk + 1) * N, :],
                            start=True, stop=True,
                            tile_position=(blk * N, blk * N))
                    usl = tbw - (M + 1)
                    nc.vector._custom_dve(
                        op3, out=redB[:], in0=tselB[:],
                        in1=uB[:, usl * N:(usl + 1) * N][:, ::-1],
                        accum_out=tags[:, tbw:tbw + 1])

            # ---------------- decode + mask + output ----------------
            # outi = (enc - 31) * (-mlt) == (31 - enc) * mlt, one 2x-rate op
            nc.vector.scalar_tensor_tensor(
                out=outi[:], in0=tags[:], scalar=31.0, in1=negmlt[:],
                op0=OP.subtract, op1=OP.mult)
            nc.sync.dma_start(out=out_d.ap(), in_=outi[:])

    nc.compile()
    return nc


def make_inputs_for_core(logits_shard, lens_shard):
    Tmat = _tmat_holder[0]
    lens = lens_shard.astype(np.int64)[:, None]
    tcolM = np.arange(M + 1)[None, :]
    tcolT = np.arange(T)[None, :]
    meq = (lens == (tcolM + 1)).astype(np.float32)
    keep = (lens - 1 != tcolT).astype(np.float32)
    mlt = (tcolT < lens).astype(np.float32)
    mrw = (tcolM >= (lens - 1)).astype(np.int8)
    irev = (31.0 - np.arange(N, dtype=np.float32))[None, :]
    irevP = np.ascontiguousarray(
        np.broadcast_to((31.0 - (np.arange(PB) % N))[:, None], (PB, N)),
        dtype=np.float32)
    rep = lambda a: np.ascontiguousarray(
        np.broadcast_to(a, (PB, a.shape[1])), dtype=np.float32)
    trepF = np.ascontiguousarray(Tmat.T).reshape(1, N * N)
    trepB = np.ascontiguousarray(Tmat).reshape(1, N * N)
    logitsB = np.ascontiguousarray(
        logits_shard[:, M:, :] - np.float32(BIAS), dtype=np.float32)
    return {
        "logits": np.ascontiguousarray(logits_shard, dtype=np.float32),
        "logitsB": logitsB,
        "trepF": rep(trepF),
        "trepB": rep(trepB),
        "meq": np.ascontiguousarray(meq, dtype=np.float32),
        "keep": np.ascontiguousarray(keep, dtype=np.float32),
        "mlt": np.ascontiguousarray(mlt, dtype=np.float32),
        "negmlt": np.ascontiguousarray(-mlt, dtype=np.float32),
        "mrw": np.ascontiguousarray(mrw, dtype=np.int8),
        "mrwf": np.ascontiguousarray(mrw, dtype=np.float32),
        "mrwc": np.ascontiguousarray(1 - mrw, dtype=np.float32),
        "irev": rep(irev),
        "irevP": irevP,
        "trevwF": np.ascontiguousarray(
            np.tile(Tmat[::-1, :].T, (4, 1)), dtype=np.float32),
        "trevwB": np.ascontiguousarray(
            np.tile(Tmat[:, ::-1], (4, 1)), dtype=np.float32),
    }


_tmat_holder = [None]


def last_exec_time_ns():
    return _last_exec_ns[0]


def kernel(logits, transitions, sequence_lengths, _trace=False):
    logits = np.asarray(logits, dtype=np.float32)
    Tmat = np.asarray(transitions, dtype=np.float32)
    lens = np.asarray(sequence_lengths)
    Bn, Tn, Nn = logits.shape
    assert Nn == N and Bn % NCORES == 0
    _tmat_holder[0] = Tmat

    if Tn not in _nc_cache:
        _nc_cache[Tn] = build_nc(Tn)
    nc = _nc_cache[Tn]

    in_maps = []
    for i in range(NCORES):
        sl = slice(i * PB, (i + 1) * PB)
        in_maps.append(make_inputs_for_core(logits[sl], lens[sl]))

    kw = {}
    if _trace:
        kw = dict(trace=True, trace_cores=[0])
    res = run_bass_kernel_spmd(nc, in_maps, core_ids=list(range(NCORES)), **kw)
    _last_exec_ns[0] = getattr(res, "exec_time_ns", None)

    out = np.concatenate([res.results[i]["out"] for i in range(NCORES)], axis=0)
    return out.astype(np.int32)


# revision 10
# speedup vs baseline: 1.0029x; 1.0006x over previous
"""CRF Viterbi decode (B=1024, T=1024, N=32) on 8 TRN2 NeuronCores — v3.

Meet-in-the-middle: a forward Viterbi recursion over t=[0, M] and a
backward (future-score) relay over t=[1023, M] run as two interleaved
DVE chains, hiding each other's dependency bubbles; the per-step logit
adds run on the Pool engine.  At t=M the two sides meet:
argmax(fwd + bwd) anchors the path, after which two independent
backtrace walks (down from M over fwd states, up from M over stored
backward u-vectors) interleave to hide the DVE->PE->DVE latency.

The backward relay's logits are biased by -BIAS (expected per-step max
gain) so backward scores stay near zero magnitude; the uniform shift
never changes an argmax but keeps f32 rounding noise ~1e-4, far below
typical decision gaps.  The forward side is bit-exact to the reference.
"""
import sys
sys.path.insert(0, "/opt/trn_rl_repo")

import numpy as np

import concourse.bass as bass
import concourse.bacc as bacc
import concourse.mybir as mybir
import concourse.tile as tile
from concourse.bass_utils import run_bass_kernel_spmd

F32 = mybir.dt.float32
I32 = mybir.dt.int32
I8 = mybir.dt.int8
AX = mybir.AxisListType
OP = mybir.AluOpType

B, T, N = 1024, 1024, 32
PB = 128
NCORES = 8
BIG = 1.0e6
M = 512           # meet point
BIAS = 2.1        # backward-relay logit bias
CH = 16           # DMA chunk (time steps)

_ops_cache = {}
_nc_cache = {}
_last_exec_ns = [None]


def register_custom_ops():
    if _ops_cache:
        return _ops_cache["BT32"]
    from concourse.dve_spec import (
        Spec, Src0, Src1, AluOp, lower, Idx, scan, Scan, MaxNeg,
    )
    from concourse.dve_ops import DveOp, OPS, has_src1
    from concourse.dve_uop import DveOpSpec, AluInp
    import concourse.dve_ops as dom

    def make(name, spec, subdim, patch=None, opcfg=None):
        for o in OPS:
            if o.name == name:
                return o
        OPS_len = len(OPS)
        dom._SUB_OPCODE_FOR_NAME[name] = dom._CUSTOM_DVE_ROW_BASE + OPS_len
        assert dom._SUB_OPCODE_FOR_NAME[name] < 0x20
        shas = {}
        for ver in ("v3", "v4"):
            uops = lower(spec, ver=ver)
            if patch is not None:
                patch(uops)
            kw = {} if opcfg is None else {"op": opcfg}
            s = DveOpSpec(name=name, opcode=dom.get_dve_sub_opcode(name),
                          uops=uops, rd1_en=has_src1(spec), **kw)
            shas[ver] = s.sha(ver)
            dom._COMPILE_CACHE[(name, ver)] = s
        op = DveOp(name, spec, subdim=subdim, uops_sha=shas)
        OPS.append(op)
        dom.CUSTOM_DVE_SPECS[name] = spec
        return op

    def make_segmax():
        """Segmented running-max of (Src0 + Src1), reset at each subdim
        (row) boundary."""
        def ref(in0, in1, c0, c1, c2):
            x = (in0 + in1).astype(np.float32)
            r = np.maximum.accumulate(x, axis=-1)
            return r, None

        sc = Scan(AluOp.MAX, Src0 + Src1, _subdim_step=MaxNeg)
        spec = Spec(body=sc, reference=ref)

        def patch(uops):
            assert len(uops) == 3, f"expected [seed, steady, step], got {len(uops)}"
            steady, step = uops[1], uops[2]
            dp = steady.datapath_config[1]
            dp.op = AluOp.MAX
            dp.alu_src0 = AluInp.CURR_ALU_OUT
            dp.alu_src1 = AluInp.PREV_ALU_OUT
            dp = step.datapath_config[1]
            dp.op = AluOp.BYPASS
            dp.alu_src0 = AluInp.PREV_ALU_OUT
            dp.alu_src1 = AluInp.PREV_ALU_OUT

        return make("CRF_SEGMAX", spec, subdim=True, patch=patch)

    FMAX = np.float32(3.4028235e38)

    def ref3(in0, in1, c0, c1, c2):
        P, K = in0.shape
        x = (in0 + in1).astype(np.float32)
        r = np.maximum.accumulate(x, axis=1)
        m = ((x == r).astype(np.float32) * np.arange(K, dtype=np.float32)[None, :])
        return m, m.max(axis=1, initial=-FMAX).reshape(P, 1)

    from concourse.dve_spec import eq
    _x3 = Src0 + Src1
    spec3 = Spec(body=eq(_x3, scan(AluOp.MAX, _x3)) * Idx, accum=AluOp.MAX,
                 reference=ref3)

    op3 = make("CRF_BT32", spec3, subdim=False)
    opF = make_segmax()

    # Fused onehot+transpose for the walk: transpose_mode applies a 32x32
    # block transpose to SRC0 only; src1 streams in the output domain
    # (verified on device).  out[32b+c, j] = (in0[32b+j, *] == in1[32b+c, j]).
    from concourse.dve_uop import OpConfig, TransposeMode

    def _blockT(x):
        o = np.empty_like(x)
        for b in range(0, x.shape[0], 32):
            o[b:b + 32, :] = x[b:b + 32, :].T
        return o

    def refT(in0, in1, c0, c1, c2):
        return (_blockT(in0) == in1).astype(np.float32), None

    specT = Spec(body=eq(Src0, Src1), reference=refT)
    opT = make("CRF_OHT2", specT, subdim=False,
               opcfg=OpConfig(transpose_mode=TransposeMode.TRANSPOSE))

    _ops_cache["BT32"] = op3
    _ops_cache["SEGMAX"] = opF
    _ops_cache["OHT2"] = opT
    return op3


def build_nc(Tn):
    assert Tn == T
    register_custom_ops()
    op3 = _ops_cache["BT32"]
    opF = _ops_cache["SEGMAX"]
    opT = _ops_cache["OHT2"]

    NFWD = M          # fwd steps t=1..M
    NBWD = T - 1 - M  # bwd segmax steps t=T-2..M (count 511)
    NU = T - 1 - M    # u slots for t in [M+1, T-1]

    nc = bacc.Bacc("TRN2", target_bir_lowering=False, debug=False,
                   num_devices=NCORES)

    logits = nc.dram_tensor("logits", [PB, T, N], F32, kind="ExternalInput")
    logitsB_d = nc.dram_tensor("logitsB", [PB, T - M, N], F32,
                               kind="ExternalInput")  # biased, slot(t)=t-M
    trepF_d = nc.dram_tensor("trepF", [PB, N * N], F32, kind="ExternalInput")
    trepB_d = nc.dram_tensor("trepB", [PB, N * N], F32, kind="ExternalInput")
    meq_d = nc.dram_tensor("meq", [PB, M + 1], F32, kind="ExternalInput")
    keep_d = nc.dram_tensor("keep", [PB, T], F32, kind="ExternalInput")
    mlt_d = nc.dram_tensor("mlt", [PB, T], F32, kind="ExternalInput")
    negmlt_d = nc.dram_tensor("negmlt", [PB, T], F32, kind="ExternalInput")
    mrw_d = nc.dram_tensor("mrw", [PB, M + 1], I8, kind="ExternalInput")
    mrwf_d = nc.dram_tensor("mrwf", [PB, M + 1], F32, kind="ExternalInput")
    mrwc_d = nc.dram_tensor("mrwc", [PB, M + 1], F32, kind="ExternalInput")
    irev_d = nc.dram_tensor("irev", [PB, N], F32, kind="ExternalInput")
    irevp_d = nc.dram_tensor("irevP", [PB, N], F32, kind="ExternalInput")
    trevwF_d = nc.dram_tensor("trevwF", [PB, N], F32, kind="ExternalInput")
    trevwB_d = nc.dram_tensor("trevwB", [PB, N], F32, kind="ExternalInput")
    out_d = nc.dram_tensor("out", [PB, T], I32, kind="ExternalOutput")

    nfch = (M + CH) // CH          # fwd chunks cover t=0..M (33, last partial)
    nbch = (T - M) // CH           # bwd chunks cover t=M..T-1 biased (32)

    with tile.TileContext(nc) as tc:
        with (
            tc.tile_pool(name="consts", bufs=1) as cpool,
            tc.tile_pool(name="states", bufs=1) as spool,
            tc.tile_pool(name="big", bufs=1) as bpool,
            tc.tile_pool(name="lchF", bufs=2) as lpoolF,
            tc.tile_pool(name="lchB", bufs=2) as lpoolB,
            tc.tile_pool(name="small", bufs=1) as mpool,
            tc.tile_pool(name="psum", bufs=1, space="PSUM") as ppool,
        ):
            trepF = cpool.tile([PB, N * N], F32, tag="trepF")
            trepB = cpool.tile([PB, N * N], F32, tag="trepB")
            meq = cpool.tile([PB, M + 1], F32, tag="meq")
            keep = cpool.tile([PB, T], F32, tag="keep")
            mlt = cpool.tile([PB, T], F32, tag="mlt")
            negmlt = cpool.tile([PB, T], F32, tag="negmlt")
            mrw = cpool.tile([PB, M + 1], I8, tag="mrw")
            mrwf = cpool.tile([PB, M + 1], F32, tag="mrwf")
            mrwc = cpool.tile([PB, M + 1], F32, tag="mrwc")
            irev = cpool.tile([PB, N], F32, tag="irev")
            irevP = cpool.tile([PB, N], F32, tag="irevP")
            trevwF = cpool.tile([PB, N], F32, tag="trevwF")
            trevwB = cpool.tile([PB, N], F32, tag="trevwB")

            statesF = spool.tile([PB, (M + 1) * N], F32, tag="statesF")
            uB = spool.tile([PB, NU * N], F32, tag="uB")

            scoresF = bpool.tile([PB, N * N], F32, tag="scoresF")
            scoresB = bpool.tile([PB, N * N], F32, tag="scoresB")
            tags = bpool.tile([PB, T], F32, tag="tags")
            outi = bpool.tile([PB, T], I32, tag="outi")
            accS = bpool.tile([PB, CH * N], F32, tag="accS")
            rsc = bpool.tile([PB, CH * N], F32, tag="rsc")
            rrw = bpool.tile([PB, 64 * N], F32, tag="rrw")

            beta0 = mpool.tile([PB, N], F32, tag="beta0")
            snap = mpool.tile([PB, N], F32, tag="snap")
            lastt = mpool.tile([PB, 1], F32, tag="lastt")
            eqs = mpool.tile([PB, N], F32, tag="eqs")
            red = mpool.tile([PB, N], F32, tag="red")
            m1 = mpool.tile([PB, 1], F32, tag="m1")
            onehot = mpool.tile([PB, N], F32, tag="onehot")
            bigoh = mpool.tile([PB, N], F32, tag="bigoh")
            mbeta = mpool.tile([PB, N], F32, tag="mbeta")
            msc = mpool.tile([PB, N], F32, tag="msc")
            onehotF2 = [mpool.tile([PB, N], F32, name=f"onehotF{x}")
                        for x in range(2)]
            onehotTF2 = [mpool.tile([PB, N], F32, name=f"onehotTF{x}")
                         for x in range(2)]
            redF = mpool.tile([PB, N], F32, tag="redF")
            onehotB2 = [mpool.tile([PB, N], F32, name=f"onehotB{x}")
                        for x in range(2)]
            onehotTB2 = [mpool.tile([PB, N], F32, name=f"onehotTB{x}")
                         for x in range(2)]
            redB = mpool.tile([PB, N], F32, tag="redB")
            tselF2 = [ppool.tile([PB, N], F32, name=f"tselF{x}")
                      for x in range(2)]
            tselB2 = [ppool.tile([PB, N], F32, name=f"tselB{x}")
                      for x in range(2)]

            # ---- first chunks gate the recursion start: issue them first ----
            nc.sync.dma_start(out=trepF[:], in_=trepF_d.ap())
            ltF0 = lpoolF.tile([PB, CH * N], F32, tag="lchF")
            nc.sync.dma_start(
                out=ltF0[:].rearrange("p (t v) -> p t v", v=N),
                in_=logits.ap()[:, 0:CH, :])
            nc.sync.dma_start(out=trepB[:], in_=trepB_d.ap())
            ltB0 = lpoolB.tile([PB, CH * N], F32, tag="lchB")
            nc.sync.dma_start(
                out=ltB0[:].rearrange("p (t v) -> p t v", v=N),
                in_=logitsB_d.ap()[:, T - M - CH:T - M, :])  # t in [1008,1023]
            nc.sync.dma_start(out=keep[:], in_=keep_d.ap())
            nc.sync.dma_start(out=meq[:], in_=meq_d.ap())
            nc.sync.dma_start(out=mlt[:], in_=mlt_d.ap())
            nc.sync.dma_start(out=negmlt[:], in_=negmlt_d.ap())
            nc.sync.dma_start(out=mrw[:], in_=mrw_d.ap())
            nc.sync.dma_start(out=mrwf[:], in_=mrwf_d.ap())
            nc.sync.dma_start(out=mrwc[:], in_=mrwc_d.ap())
            nc.sync.dma_start(out=irev[:], in_=irev_d.ap())
            nc.sync.dma_start(out=irevP[:], in_=irevp_d.ap())
            nc.sync.dma_start(out=trevwF[:], in_=trevwF_d.ap())
            nc.sync.dma_start(out=trevwB[:], in_=trevwB_d.ap())

            trepF3 = trepF[:].rearrange("p (c v) -> p c v", v=N)
            trepB3 = trepB[:].rearrange("p (c v) -> p c v", v=N)
            scF3 = scoresF[:].rearrange("p (c v) -> p c v", v=N)
            scB3 = scoresB[:].rearrange("p (c v) -> p c v", v=N)
            statesF3 = statesF[:].rearrange("p (t v) -> p t v", v=N)
            uB3 = uB[:].rearrange("p (t v) -> p t v", v=N)
            accS3 = accS[:].rearrange("p (t v) -> p t v", v=N)
            rsc3 = rsc[:].rearrange("p (t v) -> p t v", v=N)

            nc.vector.memset(beta0[:], 0.0)
            nc.gpsimd.memset(accS[:], 0.0)

            # statesF[0] = logits[0]
            nc.vector.tensor_copy(out=statesF[:, 0:N], in_=ltF0[:, 0:N])
            # u[T-1] = beta0*keep[T-1] + logitsB[T-1]  (Pool: 2x tensor_tensor;
            # TensorScalarPtr is rejected on Pool by codegen)
            keepb_last = keep[:, T - 1:T].rearrange("p (a b) -> p a b", a=1) \
                .to_broadcast((PB, N, 1))
            u_last = uB[:, (NU - 1) * N:NU * N].rearrange("p (c o) -> p c o", o=1)
            nc.gpsimd.tensor_tensor(
                out=u_last, in0=beta0[:].rearrange("p (c o) -> p c o", o=1),
                in1=keepb_last, op=OP.mult)
            nc.gpsimd.tensor_tensor(
                out=u_last, in0=u_last,
                in1=ltB0[:, (CH - 1) * N:CH * N].rearrange("p (c o) -> p c o", o=1),
                op=OP.add)

            # ---------------- phase R: interleaved recursions ----------------
            ltF, ltB = ltF0, ltB0
            ltB_next = None
            # u slot for t: t - (M+1); logitsB slot for t: t - M
            for i in range(NFWD):          # i = 0..511
                tf = i + 1                 # fwd computes statesF[tf]
                tb = T - 2 - i             # bwd computes beta_tb (i<NBWD)
                # chunk management
                if tf % CH == 0 and tf < M:
                    ltF = lpoolF.tile([PB, CH * N], F32, tag="lchF")
                    nc.sync.dma_start(
                        out=ltF[:].rearrange("p (t v) -> p t v", v=N),
                        in_=logits.ap()[:, tf:tf + CH, :])
                elif tf == M:
                    ltF = lpoolF.tile([PB, CH * N], F32, tag="lchF")
                    nc.sync.dma_start(
                        out=ltF[:, 0:N].rearrange("p (t v) -> p t v", v=N),
                        in_=logits.ap()[:, M:M + 1, :])
                # bwd chunk k = i//CH + 1 prefetched a full chunk early
                # (issued at i ≡ 0 mod CH, first consumed at i ≡ CH-1)
                if i % CH == 0 and i // CH + 1 < nbch:
                    k = i // CH + 1
                    ltB_next = lpoolB.tile([PB, CH * N], F32, tag="lchB")
                    nc.sync.dma_start(
                        out=ltB_next[:].rearrange("p (t v) -> p t v", v=N),
                        in_=logitsB_d.ap()[:, T - M - CH * (k + 1):
                                           T - M - CH * k, :])
                if (i + 1) % CH == 0:
                    # tb at this i (= T-CH*k-1) already reads the new chunk
                    ltB = ltB_next

                # ---- fwd step ----
                sprev_b = statesF[:, (tf - 1) * N:tf * N] \
                    .rearrange("p (o v) -> p o v", o=1).to_broadcast((PB, N, N))
                nc.vector._custom_dve(opF, out=scF3, in0=sprev_b, in1=trepF3)
                nc.gpsimd.tensor_tensor(
                    out=statesF[:, tf * N:(tf + 1) * N]
                        .rearrange("p (c o) -> p c o", o=1),
                    in0=scF3[:, :, N - 1:N],
                    in1=ltF[:, (tf % CH) * N:(tf % CH + 1) * N]
                        .rearrange("p (c o) -> p c o", o=1),
                    op=OP.add)

                # ---- bwd step ----
                if i < NBWD:
                    usl1 = tb + 1 - (M + 1)
                    u_next_b = uB[:, usl1 * N:(usl1 + 1) * N] \
                        .rearrange("p (o v) -> p o v", o=1) \
                        .to_broadcast((PB, N, N))
                    nc.vector._custom_dve(opF, out=scB3, in0=u_next_b,
                                          in1=trepB3)
                    if tb >= M + 1:
                        usl0 = tb - (M + 1)
                        u_out = uB[:, usl0 * N:(usl0 + 1) * N] \
                            .rearrange("p (c o) -> p c o", o=1)
                        keepb = keep[:, tb:tb + 1] \
                            .rearrange("p (a b) -> p a b", a=1) \
                            .to_broadcast((PB, N, 1))
                        nc.gpsimd.tensor_tensor(
                            out=u_out, in0=scB3[:, :, N - 1:N], in1=keepb,
                            op=OP.mult)
                        nc.gpsimd.tensor_tensor(
                            out=u_out, in0=u_out,
                            in1=ltB[:, (tb % CH) * N:(tb % CH + 1) * N]
                                .rearrange("p (c o) -> p c o", o=1),
                            op=OP.add)

                # late rows [M-CH, M-4): accumulate as soon as written so
                # only a 4+1-row tail gates the meet
                if tf >= M - CH + 5 and (tf - (M - CH + 5)) % 4 == 0 \
                        and tf <= M - 3:
                    t0l = tf - 5
                    rl = (t0l - (M - CH)) // 4
                    meqbl = meq[:, t0l:t0l + 4] \
                        .rearrange("p (t o) -> p t o", o=1) \
                        .to_broadcast((PB, 4, N))
                    nc.gpsimd.tensor_tensor(
                        out=rsc3[:, rl * 4:rl * 4 + 4, :],
                        in0=statesF3[:, t0l:t0l + 4, :], in1=meqbl,
                        op=OP.mult)
                    nc.gpsimd.tensor_tensor(
                        out=accS3[:, rl * 4:rl * 4 + 4, :],
                        in0=accS3[:, rl * 4:rl * 4 + 4, :],
                        in1=rsc3[:, rl * 4:rl * 4 + 4, :], op=OP.add)
                # ---- snapshot accumulation (Pool), spread in 4-row pieces
                # one chunk behind the recursion to avoid queue bursts ----
                if tf > CH and (tf - 1) % 4 == 0:
                    # piece r of chunk ending at the last CH boundary
                    bnd = ((tf - 1) // CH) * CH     # last boundary < tf
                    r = ((tf - 1) % CH) // 4
                    t0 = bnd - CH + r * 4
                    if t0 + 4 <= bnd:
                        meqb = meq[:, t0:t0 + 4] \
                            .rearrange("p (t o) -> p t o", o=1) \
                            .to_broadcast((PB, 4, N))
                        nc.gpsimd.tensor_tensor(
                            out=rsc3[:, r * 4:r * 4 + 4, :],
                            in0=statesF3[:, t0:t0 + 4, :], in1=meqb,
                            op=OP.mult)
                        nc.gpsimd.tensor_tensor(
                            out=accS3[:, r * 4:r * 4 + 4, :],
                            in0=accS3[:, r * 4:r * 4 + 4, :],
                            in1=rsc3[:, r * 4:r * 4 + 4, :], op=OP.add)

            # remaining snapshot rows: in-loop pieces covered [0, M-4);
            # add [M-4, M] (5 rows) here.
            meqb_m = meq[:, M - 4:M + 1].rearrange("p (t o) -> p t o", o=1) \
                .to_broadcast((PB, 5, N))
            rscw = rsc[:].rearrange("p (t v) -> p t v", v=N)
            accw = accS[:].rearrange("p (t v) -> p t v", v=N)
            nc.gpsimd.tensor_tensor(
                out=rscw[:, 0:4, :], in0=statesF3[:, M - 4:M, :],
                in1=meqb_m[:, 0:4, :], op=OP.mult)
            nc.gpsimd.tensor_tensor(
                out=accw[:, 0:4, :], in0=accw[:, 0:4, :],
                in1=rscw[:, 0:4, :], op=OP.add)
            nc.gpsimd.tensor_tensor(
                out=rscw[:, 0:1, :], in0=statesF3[:, M:M + 1, :],
                in1=meqb_m[:, 4:5, :], op=OP.mult)
            nc.gpsimd.tensor_tensor(
                out=accw[:, 0:1, :], in0=accw[:, 0:1, :],
                in1=rscw[:, 0:1, :], op=OP.add)

            # ---------------- phase M: snap -> last_tag -> rewrite -> meet ----
            acc_vt = accS[:].rearrange("p (t v) -> p v t", v=N)
            nc.vector.tensor_reduce(out=snap[:], in_=acc_vt, axis=AX.X,
                                    op=OP.add)
            nc.vector._custom_dve(
                op3, out=red[:], in0=snap[:, ::-1],
                in1=beta0[:], accum_out=lastt[:])

            # BIG-rewrite of statesF rows t in [len-1, M]; top rows sync via
            # copy_predicated, lower rows async on Pool during the walk.
            RS = M - 64
            RCH = 64
            nc.vector.tensor_scalar(
                out=onehot[:], in0=irev[:], scalar1=lastt[:], scalar2=None,
                op0=OP.is_equal)
            nc.vector.tensor_scalar(
                out=bigoh[:], in0=onehot[:], scalar1=BIG, scalar2=None,
                op0=OP.mult)
            # rows [M-16, M] first: they gate the meet and the walk's start;
            # the rest follows while the meet chain runs
            RS2 = M - 16
            mrwb2 = mrw[:, RS2:M + 1].rearrange("p (t o) -> p t o", o=1) \
                .to_broadcast((PB, M + 1 - RS2, N))
            bigohb2 = bigoh[:].rearrange("p (o v) -> p o v", o=1) \
                .to_broadcast((PB, M + 1 - RS2, N))
            nc.vector.copy_predicated(out=statesF3[:, RS2:M + 1, :],
                                      mask=mrwb2, data=bigohb2)
            # meet: mbeta + anchor argmax (needs only row M)
            nc.vector.tensor_scalar(
                out=mbeta[:].rearrange("p (c o) -> p c o", o=1),
                in0=scB3[:, :, N - 1:N],
                scalar1=keep[:, M:M + 1], scalar2=None, op0=OP.mult)
            nc.vector._custom_dve(
                op3, out=red[:], in0=statesF[:, M * N:(M + 1) * N][:, ::-1],
                in1=mbeta[:, ::-1], accum_out=tags[:, M:M + 1])
            mrwb = mrw[:, RS:RS2].rearrange("p (t o) -> p t o", o=1) \
                .to_broadcast((PB, RS2 - RS, N))
            bigohb = bigoh[:].rearrange("p (o v) -> p o v", o=1) \
                .to_broadcast((PB, RS2 - RS, N))
            nc.vector.copy_predicated(out=statesF3[:, RS:RS2, :], mask=mrwb,
                                      data=bigohb)
            rrw3 = rrw[:].rearrange("p (t v) -> p t v", v=N)
            bigohc = bigoh[:].rearrange("p (o v) -> p o v", o=1) \
                .to_broadcast((PB, RCH, N))
            for t0 in range(RS - RCH, -1, -RCH):
                stc = statesF3[:, t0:t0 + RCH, :]
                mc = mrwc[:, t0:t0 + RCH] \
                    .rearrange("p (t o) -> p t o", o=1).to_broadcast((PB, RCH, N))
                mf = mrwf[:, t0:t0 + RCH] \
                    .rearrange("p (t o) -> p t o", o=1).to_broadcast((PB, RCH, N))
                nc.gpsimd.tensor_tensor(out=stc, in0=stc, in1=mc, op=OP.mult)
                nc.gpsimd.tensor_tensor(out=rrw3, in0=bigohc, in1=mf,
                                        op=OP.mult)
                nc.gpsimd.tensor_tensor(out=stc, in0=stc, in1=rrw3, op=OP.add)

            # ---------------- phase W: two interleaved walks ----------------
            for j in range(M):
                tfw = M - 1 - j            # fwd walk target position
                tbw = M + 1 + j            # bwd walk target position
                par = j % 2
                onehotF, onehotTF, tselF = (onehotF2[par], onehotTF2[par],
                                            tselF2[par])
                onehotB, onehotTB, tselB = (onehotB2[par], onehotTB2[par],
                                            tselB2[par])
                # fwd walk step: fused compare + src0 block-transpose
                nc.vector._custom_dve(
                    opT, out=onehotTF[:],
                    in0=tags[:, tfw + 1:tfw + 2].to_broadcast((PB, N)),
                    in1=irevP[:])
                for blk in range(4):
                    nc.tensor.matmul(
                        out=tselF[blk * N:(blk + 1) * N, :],
                        lhsT=onehotTF[blk * N:(blk + 1) * N, :],
                        rhs=trevwF[blk * N:(blk + 1) * N, :],
                        start=True, stop=True,
                        tile_position=(blk * N, blk * N))
                nc.vector._custom_dve(
                    op3, out=redF[:], in0=tselF[:],
                    in1=statesF[:, tfw * N:(tfw + 1) * N][:, ::-1],
                    accum_out=tags[:, tfw:tfw + 1])
                # bwd walk step
                if tbw <= T - 1:
                    nc.vector._custom_dve(
                        opT, out=onehotTB[:],
                        in0=tags[:, tbw - 1:tbw].to_broadcast((PB, N)),
                        in1=irevP[:])
                    for blk in range(4):
                        nc.tensor.matmul(
                            out=tselB[blk * N:(blk + 1) * N, :],
                            lhsT=onehotTB[blk * N:(blk + 1) * N, :],
                            rhs=trevwB[blk * N:(blk + 1) * N, :],
                            start=True, stop=True,
                            tile_position=(blk * N, blk * N))
                    usl = tbw - (M + 1)
                    nc.vector._custom_dve(
                        op3, out=redB[:], in0=tselB[:],
                        in1=uB[:, usl * N:(usl + 1) * N][:, ::-1],
                        accum_out=tags[:, tbw:tbw + 1])

            # ---------------- decode + mask + output ----------------
            # outi = (enc - 31) * (-mlt) == (31 - enc) * mlt, one 2x-rate op;
            # two halves so the first DMA overlaps the second decode
            H = T // 2
            nc.vector.scalar_tensor_tensor(
                out=outi[:, 0:H], in0=tags[:, 0:H], scalar=31.0,
                in1=negmlt[:, 0:H], op0=OP.subtract, op1=OP.mult)
            nc.sync.dma_start(out=out_d.ap()[:, 0:H], in_=outi[:, 0:H])
            nc.vector.scalar_tensor_tensor(
                out=outi[:, H:T], in0=tags[:, H:T], scalar=31.0,
                in1=negmlt[:, H:T], op0=OP.subtract, op1=OP.mult)
            nc.sync.dma_start(out=out_d.ap()[:, H:T], in_=outi[:, H:T])

    nc.compile()
    return nc


def make_inputs_for_core(logits_shard, lens_shard):
    Tmat = _tmat_holder[0]
    lens = lens_shard.astype(np.int64)[:, None]
    tcolM = np.arange(M + 1)[None, :]
    tcolT = np.arange(T)[None, :]
    meq = (lens == (tcolM + 1)).astype(np.float32)
    keep = (lens - 1 != tcolT).astype(np.float32)
    mlt = (tcolT < lens).astype(np.float32)
    mrw = (tcolM >= (lens - 1)).astype(np.int8)
    irev = (31.0 - np.arange(N, dtype=np.float32))[None, :]
    irevP = np.ascontiguousarray(
        np.broadcast_to((31.0 - (np.arange(PB) % N))[:, None], (PB, N)),
        dtype=np.float32)
    rep = lambda a: np.ascontiguousarray(
        np.broadcast_to(a, (PB, a.shape[1])), dtype=np.float32)
    trepF = np.ascontiguousarray(Tmat.T).reshape(1, N * N)
    trepB = np.ascontiguousarray(Tmat).reshape(1, N * N)
    logitsB = np.ascontiguousarray(
        logits_shard[:, M:, :] - np.float32(BIAS), dtype=np.float32)
    return {
        "logits": np.ascontiguousarray(logits_shard, dtype=np.float32),
        "logitsB": logitsB,
        "trepF": rep(trepF),
        "trepB": rep(trepB),
        "meq": np.ascontiguousarray(meq, dtype=np.float32),
        "keep": np.ascontiguousarray(keep, dtype=np.float32),
        "mlt": np.ascontiguousarray(mlt, dtype=np.float32),
        "negmlt": np.ascontiguousarray(-mlt, dtype=np.float32),
        "mrw": np.ascontiguousarray(mrw, dtype=np.int8),
        "mrwf": np.ascontiguousarray(mrw, dtype=np.float32),
        "mrwc": np.ascontiguousarray(1 - mrw, dtype=np.float32),
        "irev": rep(irev),
        "irevP": irevP,
        "trevwF": np.ascontiguousarray(
            np.tile(Tmat[::-1, :].T, (4, 1)), dtype=np.float32),
        "trevwB": np.ascontiguousarray(
            np.tile(Tmat[:, ::-1], (4, 1)), dtype=np.float32),
    }


_tmat_holder = [None]


def last_exec_time_ns():
    return _last_exec_ns[0]


def kernel(logits, transitions, sequence_lengths, _trace=False):
    logits = np.asarray(logits, dtype=np.float32)
    Tmat = np.asarray(transitions, dtype=np.float32)
    lens = np.asarray(sequence_lengths)
    Bn, Tn, Nn = logits.shape
    assert Nn == N and Bn % NCORES == 0
    _tmat_holder[0] = Tmat

    if Tn not in _nc_cache:
        _nc_cache[Tn] = build_nc(Tn)
    nc = _nc_cache[Tn]

    in_maps = []
    for i in range(NCORES):
        sl = slice(i * PB, (i + 1) * PB)
        in_maps.append(make_inputs_for_core(logits[sl], lens[sl]))

    kw = {}
    if _trace:
        kw = dict(trace=True, trace_cores=[0])
    res = run_bass_kernel_spmd(nc, in_maps, core_ids=list(range(NCORES)), **kw)
    _last_exec_ns[0] = getattr(res, "exec_time_ns", None)

    out = np.concatenate([res.results[i]["out"] for i in range(NCORES)], axis=0)
    return out.astype(np.int32)
